# revision 1
# baseline (speedup 1.0000x reference)
"""Trainium2 Bass kernel for nn_BDLModel (gnn_message_passing).

Strategy (8 NeuronCores, SPMD):
  - Nodes sharded contiguously across cores (3750/core); edges partitioned by
    dst owner and sorted by dst; per dst-tile (128 nodes) edge lists padded to
    chunks of 128 edges.
  - Mean aggregation: AllGather the activation table to DRAM (bf16), then per
    dst-tile one dma_gather pulls all source rows ([128, CK, W] edge-major),
    and per 128-edge chunk a one-hot selection matrix S[e,d] (generated
    on-chip by comparing an iota row tile against per-edge dst-local ids) is
    the stationary matmul operand: psum[d, :] += S^T @ Zgathered. 1/deg is
    applied during PSUM evacuation (per-partition tensor_scalar).
  - The Householder orthogonalization for D=2 has the closed form
    Q = [[c, s], [-s, c]], c=(a^2-1)/(1+a^2), s=2a/(1+a^2) where a is the
    strict-lower entry (column 2 of each 4-wide block of nr). Only the
    2::4 columns of enc_w2 are ever needed.
  - All activations node-major [128 nodes, W]; X@W matmuls use PE-transposed
    activation chunks as the stationary operand with natural [K, M] weight
    slices streaming, so outputs stay node-major.

Self-contained: hardcodes shapes, only imports numpy + the concourse stack.
"""

import math
import os
import sys
from dataclasses import dataclass

import numpy as np

for _p in ("/opt/trn_rl_repo", "/root/.axon_site/_ro/trn_rl_repo"):
    if os.path.isdir(_p) and _p not in sys.path:
        sys.path.insert(0, _p)

import ml_dtypes  # noqa: E402

BF16 = ml_dtypes.bfloat16


@dataclass(frozen=True)
class Cfg:
    N: int = 30000
    E: int = 480000
    HID: int = 256
    NB: int = 128
    D: int = 2
    NL: int = 2
    NSAGE: int = 5
    OUT: int = 5
    NC: int = 8
    EPS: float = 1e-5

    @property
    def SW(self):
        return self.D * self.D * self.NB

    @property
    def NLOC(self):
        return self.N // self.NC

    @property
    def NT(self):
        return (self.NLOC + 127) // 128


CFG = Cfg()


# ---------------------------------------------------------------- host prep


def _prep_rhs(w):
    """[K, M] -> [128, (K//128)*M] so slice kc -> [:, kc*M:(kc+1)*M] = W[kc]."""
    k, m = w.shape
    assert k % 128 == 0
    kc = k // 128
    return np.ascontiguousarray(
        w.reshape(kc, 128, m).transpose(1, 0, 2).reshape(128, kc * m)
    ).astype(BF16)


def _prep_bias(b):
    return np.ascontiguousarray(np.tile(np.asarray(b, np.float32).reshape(1, -1), (128, 1)))


def _prep_edges(cfg: Cfg, edge_index):
    """Partition edges by dst owner; per dst-tile padded chunk schedule."""
    src = np.asarray(edge_index[0], np.int64)
    dst = np.asarray(edge_index[1], np.int64)
    deg = np.bincount(dst, minlength=cfg.N).astype(np.float64)
    rdeg_full = (1.0 / np.maximum(deg, 1.0)).astype(np.float32)

    per_core = []
    ck_max = 1
    for c in range(cfg.NC):
        lo, hi = c * cfg.NLOC, (c + 1) * cfg.NLOC
        m = (dst >= lo) & (dst < hi)
        s_c = src[m]
        d_c = dst[m] - lo
        order = np.argsort(d_c, kind="stable")
        s_c, d_c = s_c[order], d_c[order]
        bounds = np.searchsorted(d_c, np.arange(cfg.NT + 1) * 128)
        cnts = bounds[1:] - bounds[:-1]
        ck_c = max(1, int(math.ceil(cnts.max() / 128))) if len(s_c) else 1
        ck_max = max(ck_max, ck_c)
        per_core.append((s_c, d_c, bounds))

    CK = ck_max
    outs = []
    for c in range(cfg.NC):
        s_c, d_c, bounds = per_core[c]
        idx16 = np.zeros((128, cfg.NT * CK * 8), np.int16)
        ids = np.full((128, cfg.NT * CK), 255.0, BF16)
        for t in range(cfg.NT):
            b0, b1 = bounds[t], bounds[t + 1]
            n = b1 - b0
            if n == 0:
                continue
            i = np.arange(n)
            # gather order: unwrapped[i] = idx16[i%16, i//16] (replicated x8)
            col = t * CK * 8 + i // 16
            row = i % 16
            for g in range(8):
                idx16[row + 16 * g, col] = s_c[b0:b1]
            ids[i % 128, t * CK + i // 128] = (d_c[b0:b1] - t * 128).astype(BF16)
        rdeg = np.ones((128, cfg.NT), np.float32)
        nval = cfg.NLOC
        rfull = rdeg_full[c * cfg.NLOC : (c + 1) * cfg.NLOC]
        for t in range(cfg.NT):
            r0 = t * 128
            nr = min(128, nval - r0)
            rdeg[:nr, t] = rfull[r0 : r0 + nr]
        outs.append(dict(idx16=idx16, ids=ids, rdeg=rdeg))
    return CK, outs


def _prep_inputs(cfg: Cfg, inputs):
    """Build the per-core in_maps. Returns (CK, in_maps)."""
    f32 = np.float32
    x = np.asarray(inputs["x"], f32)
    CK, edge_outs = _prep_edges(cfg, np.asarray(inputs["edge_index"]))

    g = lambda k: np.asarray(inputs[k], f32)

    shared = {
        "w_in_r": _prep_rhs(g("w_in")),
        "b_in_bc": _prep_bias(g("b_in")),
        "w_si_r": _prep_rhs(g("se_in_w")),
        "b_si_bc": _prep_bias(g("se_in_b")),
        "w_s1_r": np.concatenate([_prep_rhs(g("sage_w1")[i]) for i in range(cfg.NSAGE)], axis=1),
        "b_s1_bc": np.concatenate([_prep_bias(g("sage_b1")[i]) for i in range(cfg.NSAGE)], axis=1),
        "w_s2_r": np.concatenate([_prep_rhs(g("sage_w2")[i]) for i in range(cfg.NSAGE)], axis=1),
        "b_s2_bc": np.concatenate([_prep_bias(g("sage_b2")[i]) for i in range(cfg.NSAGE)], axis=1),
        "w_so_r": _prep_rhs(g("se_out_w")),
        "b_so_bc": _prep_bias(g("se_out_b")),
        "w_e1_r": np.concatenate([_prep_rhs(g("enc_w1")[k]) for k in range(cfg.NL)], axis=1),
        "b_e1_bc": np.concatenate([_prep_bias(g("enc_b1")[k]) for k in range(cfg.NL)], axis=1),
        "w_e2_r": np.concatenate(
            [_prep_rhs(np.ascontiguousarray(g("enc_w2")[k][:, 2::4])) for k in range(cfg.NL)], axis=1
        ),
        "b_e2_bc": np.concatenate([_prep_bias(g("enc_b2")[k][2::4]) for k in range(cfg.NL)], axis=1),
        "ln_g_bc": np.concatenate([_prep_bias(g("ln_g")[k]) for k in range(cfg.NL)], axis=1),
        "ln_b_bc": np.concatenate([_prep_bias(g("ln_b")[k]) for k in range(cfg.NL)], axis=1),
        "w_b1_r": np.concatenate([_prep_rhs(g("bdl_w1")[k]) for k in range(cfg.NL)], axis=1),
        "b_b1_bc": np.concatenate([_prep_bias(g("bdl_b1")[k]) for k in range(cfg.NL)], axis=1),
        "w_b2_r": np.concatenate([_prep_rhs(g("bdl_w2")[k]) for k in range(cfg.NL)], axis=1),
        "b_b2_bc": np.concatenate([_prep_bias(g("bdl_b2")[k]) for k in range(cfg.NL)], axis=1),
        "oln_g_bc": _prep_bias(g("out_ln_g")),
        "oln_b_bc": _prep_bias(g("out_ln_b")),
        "w_o_r": _prep_rhs(g("w_out")),
        "b_o_bc": _prep_bias(g("b_out")),
        "ident_f": np.eye(128, dtype=f32),
        "ident_b": np.eye(128, dtype=BF16),
        "iota_f": np.tile(np.arange(128), (128, 1)).astype(BF16),
    }

    in_maps = []
    for c in range(cfg.NC):
        m = dict(shared)
        m["x_c"] = np.ascontiguousarray(x[c * cfg.NLOC : (c + 1) * cfg.NLOC])
        m["idx16"] = edge_outs[c]["idx16"]
        m["ids_f"] = edge_outs[c]["ids"]
        m["rdeg"] = edge_outs[c]["rdeg"]
        in_maps.append(m)
    return CK, in_maps


# ---------------------------------------------------------------- builder


def build_program(cfg: Cfg, CK: int):
    from concourse import bacc, mybir
    import concourse.tile as tile

    f32 = mybir.dt.float32
    bf16 = mybir.dt.bfloat16
    i16 = mybir.dt.int16
    ALU = mybir.AluOpType
    AX = mybir.AxisListType
    ACT = mybir.ActivationFunctionType

    NT, NLOC, HID, SW = cfg.NT, cfg.NLOC, cfg.HID, cfg.SW

    nc = bacc.Bacc(
        "TRN2",
        target_bir_lowering=False,
        debug=False,
        enable_asserts=False,
        num_devices=cfg.NC,
        num_swdge_queues=2,
    )
    rg = [list(range(cfg.NC))]

    # ---- external I/O
    d_x = nc.dram_tensor("x_c", [NLOC, HID], f32, kind="ExternalInput").ap()
    d_idx = nc.dram_tensor("idx16", [128, NT * CK * 8], i16, kind="ExternalInput").ap()
    d_ids = nc.dram_tensor("ids_f", [128, NT * CK], bf16, kind="ExternalInput").ap()
    d_rdeg = nc.dram_tensor("rdeg", [128, NT], f32, kind="ExternalInput").ap()

    def din(name, shape, dt):
        return nc.dram_tensor(name, shape, dt, kind="ExternalInput").ap()

    NS, NL = cfg.NSAGE, cfg.NL
    d_w_in = din("w_in_r", [128, 2 * HID], bf16)
    d_b_in = din("b_in_bc", [128, HID], f32)
    d_w_si = din("w_si_r", [128, 2 * SW], bf16)
    d_b_si = din("b_si_bc", [128, SW], f32)
    d_w_s1 = din("w_s1_r", [128, NS * 8 * SW], bf16)
    d_b_s1 = din("b_s1_bc", [128, NS * SW], f32)
    d_w_s2 = din("w_s2_r", [128, NS * 4 * SW], bf16)
    d_b_s2 = din("b_s2_bc", [128, NS * SW], f32)
    d_w_so = din("w_so_r", [128, 4 * SW], bf16)
    d_b_so = din("b_so_bc", [128, SW], f32)
    d_w_e1 = din("w_e1_r", [128, NL * 4 * SW], bf16)
    d_b_e1 = din("b_e1_bc", [128, NL * SW], f32)
    d_w_e2 = din("w_e2_r", [128, NL * 4 * 128], bf16)
    d_b_e2 = din("b_e2_bc", [128, NL * 128], f32)
    d_ln_g = din("ln_g_bc", [128, NL * HID], f32)
    d_ln_b = din("ln_b_bc", [128, NL * HID], f32)
    d_w_b1 = din("w_b1_r", [128, NL * 4 * HID], bf16)
    d_b_b1 = din("b_b1_bc", [128, NL * HID], f32)
    d_w_b2 = din("w_b2_r", [128, NL * 2 * HID], bf16)
    d_b_b2 = din("b_b2_bc", [128, NL * HID], f32)
    d_oln_g = din("oln_g_bc", [128, HID], f32)
    d_oln_b = din("oln_b_bc", [128, HID], f32)
    d_w_o = din("w_o_r", [128, 2 * cfg.OUT], bf16)
    d_b_o = din("b_o_bc", [128, cfg.OUT], f32)
    d_identf = din("ident_f", [128, 128], f32)
    d_identb = din("ident_b", [128, 128], bf16)
    d_iota = din("iota_f", [128, 128], bf16)

    d_out = nc.dram_tensor("out", [NLOC, cfg.OUT], f32, kind="ExternalOutput").ap()

    with tile.TileContext(nc) as tc:
        from contextlib import ExitStack

        ctx = ExitStack()
        pers = ctx.enter_context(tc.tile_pool(name="pers", bufs=1))
        wout = ctx.enter_context(tc.tile_pool(name="wout", bufs=1))
        wsage = ctx.enter_context(tc.tile_pool(name="wsage", bufs=1))
        work = ctx.enter_context(tc.tile_pool(name="work", bufs=2))
        small = ctx.enter_context(tc.tile_pool(name="small", bufs=2))
        spool = ctx.enter_context(tc.tile_pool(name="spool", bufs=2))
        zgp = ctx.enter_context(tc.tile_pool(name="zgp", bufs=2))
        psum = ctx.enter_context(tc.tile_pool(name="psum", bufs=2, space="PSUM"))
        dram = ctx.enter_context(tc.tile_pool(name="dram", bufs=1, space="DRAM"))

        # ---- persistent SBUF residents
        identf = pers.tile([128, 128], f32, name="identf")
        identb = pers.tile([128, 128], bf16, name="identb")
        iota = pers.tile([128, 128], bf16, name="iota")
        rdeg = pers.tile([128, NT], f32, name="rdegs")
        ids = pers.tile([128, NT * CK], bf16, name="idss")
        idx = pers.tile([128, NT * CK * 8], i16, name="idxs")
        nc.sync.dma_start(out=identf[:], in_=d_identf[:])
        nc.sync.dma_start(out=identb[:], in_=d_identb[:])
        nc.sync.dma_start(out=iota[:], in_=d_iota[:])
        nc.sync.dma_start(out=rdeg[:], in_=d_rdeg[:])
        nc.sync.dma_start(out=ids[:], in_=d_ids[:])
        nc.sync.dma_start(out=idx[:], in_=d_idx[:])

        h_t = [pers.tile([128, HID], f32, name=f"h{t}") for t in range(NT)]
        c_t = [pers.tile([128, 128], bf16, name=f"rc{t}") for t in range(NT)]
        s_t = [pers.tile([128, 128], bf16, name=f"rs{t}") for t in range(NT)]

        # ---- per-outer-layer weights (small, reloaded per outer layer)
        def load_w(pool, name, src, cols, dt):
            t = pool.tile([128, cols], dt, tag=name, name=name)
            nc.sync.dma_start(out=t[:], in_=src)
            return t

        # DRAM scratch
        def dram_tile(name, shape, dt, shared=False):
            return dram.tile(
                shape, dt, tag=name, name=name, addr_space="Shared" if shared else "Local"
            )

        def rows_of(t):
            return min(128, NLOC - t * 128)

        # ---------- helpers ----------
        def transpose_into(dst, src_ap, nchunks, is_f32):
            """dst[:, kc*128:(kc+1)*128] = src[:, kc*128:(kc+1)*128]^T (bf16 out)."""
            for kc in range(nchunks):
                if is_f32:
                    tp = psum.tile([128, 128], f32, tag="tr", name="trf")
                    nc.tensor.transpose(
                        tp[:], src_ap[:, kc * 128 : (kc + 1) * 128], identf[:]
                    )
                else:
                    tp = psum.tile([128, 128], bf16, tag="tr", name="trb")
                    nc.tensor.transpose(
                        tp[:], src_ap[:, kc * 128 : (kc + 1) * 128], identb[:]
                    )
                nc.vector.tensor_copy(out=dst[:, kc * 128 : (kc + 1) * 128], in_=tp[:])

        def mm_acc(ps_ap, lhsT_tile, rhs_tile, kcs, m, rhs_block):
            """ps += sum_kc lhsT[:, kc]^T @ rhs[:, kc-block] (node-major out)."""
            for kc in range(kcs):
                nc.tensor.matmul(
                    ps_ap,
                    lhsT=lhsT_tile[:, kc * 128 : (kc + 1) * 128],
                    rhs=rhs_tile[:, kc * rhs_block + m[0] : kc * rhs_block + m[1]],
                    start=(kc == 0),
                    stop=(kc == kcs - 1),
                )

        def emit_ln(h_ap, g_bc_ap, b_bc_ap, out_ap, w):
            s1 = small.tile([128, 1], f32, tag="ln1", name="ln1")
            nc.vector.reduce_sum(out=s1[:], in_=h_ap, axis=AX.X)
            nm = small.tile([128, 1], f32, tag="ln2", name="ln2")
            nc.vector.tensor_scalar(nm[:], s1[:], -1.0 / w, None, ALU.mult)
            cen = work.tile([128, w], f32, tag="lncen", name="lncen")
            nc.vector.tensor_scalar(cen[:], h_ap, nm[:], None, ALU.add)
            sq = work.tile([128, w], f32, tag="tmpf", name="lnsq")
            nc.vector.tensor_tensor(out=sq[:], in0=cen[:], in1=cen[:], op=ALU.mult)
            v = small.tile([128, 1], f32, tag="ln3", name="ln3")
            nc.vector.reduce_sum(out=v[:], in_=sq[:], axis=AX.X)
            vm = small.tile([128, 1], f32, tag="ln4", name="ln4")
            nc.vector.tensor_scalar(vm[:], v[:], 1.0 / w, cfg.EPS, ALU.mult, ALU.add)
            r = small.tile([128, 1], f32, tag="ln5", name="ln5")
            nc.vector.reciprocal(out=r[:], in_=vm[:])
            rs = small.tile([128, 1], f32, tag="ln6", name="ln6")
            nc.scalar.sqrt(out=rs[:], in_=r[:])
            nc.vector.tensor_scalar(cen[:], cen[:], rs[:], None, ALU.mult)
            nc.vector.tensor_tensor(out=cen[:], in0=cen[:], in1=g_bc_ap, op=ALU.mult)
            nc.vector.tensor_tensor(out=out_ap, in0=cen[:], in1=b_bc_ap, op=ALU.add)

        def emit_agg(table_ap, t, width, ps_ap):
            """Gather + one-hot matmul segment sum for dst tile t into psum."""
            zg = zgp.tile([128, CK, width], bf16, tag="zg", name="zg")
            cka = (CK + 1) // 2
            for qi, (c0, c1) in enumerate(((0, cka), (cka, CK))):
                if c1 > c0:
                    nc.gpsimd.dma_gather(
                        out_ap=zg[:, c0:c1, :],
                        in_ap=table_ap,
                        idxs_ap=idx[:, t * CK * 8 + c0 * 8 : t * CK * 8 + c1 * 8],
                        num_idxs=(c1 - c0) * 128,
                        num_idxs_reg=(c1 - c0) * 128,
                        elem_size=width,
                        single_packet=False,
                        queue_num=qi,
                    )
            Sall = spool.tile([128, CK * 128], bf16, tag="S", name="S")
            iota_bc = iota[:].rearrange("p (o f) -> p o f", o=1).to_broadcast([128, CK, 128])
            ids_bc = (
                ids[:, t * CK : (t + 1) * CK]
                .rearrange("p (c o) -> p c o", o=1)
                .to_broadcast([128, CK, 128])
            )
            nc.vector.tensor_tensor(out=Sall[:], in0=iota_bc, in1=ids_bc, op=ALU.is_equal)
            for c in range(CK):
                nc.tensor.matmul(
                    ps_ap,
                    lhsT=Sall[:, c * 128 : (c + 1) * 128],
                    rhs=zg[:, c, :],
                    start=(c == 0),
                    stop=(c == CK - 1),
                )

        # ================= phase 0: h0 = gelu(x @ w_in + b_in) =================
        w_in_sb = load_w(wout, "w_in", d_w_in[:], 2 * HID, bf16)
        b_in_sb = load_w(wout, "b_in", d_b_in[:], HID, f32)
        w_si_sb = load_w(wout, "w_si", d_w_si[:], 2 * SW, bf16)
        b_si_sb = load_w(wout, "b_si", d_b_si[:], SW, f32)

        ph0 = tc.tile_pool(name="ph0", bufs=2)
        ph0ctx = ph0.__enter__()
        for t in range(NT):
            nr = rows_of(t)
            xt = ph0ctx.tile([128, HID], f32, tag="xt", name="xt")
            if nr < 128:
                nc.gpsimd.memset(xt[:], 0.0)
            nc.sync.dma_start(out=xt[:nr, :], in_=d_x[t * 128 : t * 128 + nr, :])
            xT = ph0ctx.tile([128, 2 * 128], bf16, tag="xT", name="xT")
            transpose_into(xT, xt[:], 2, True)
            hp = psum.tile([128, HID], f32, tag="mlp", name="hp")
            mm_acc(hp[:], xT, w_in_sb, 2, (0, HID), HID)
            hpre = work.tile([128, HID], f32, tag="tmpf", name="hpre")
            nc.vector.tensor_tensor(out=hpre[:], in0=hp[:], in1=b_in_sb[:], op=ALU.add)
            nc.scalar.activation(out=h_t[t][:], in_=hpre[:], func=ACT.Gelu)
        ph0.__exit__(None, None, None)

        # ================= outer layers =================
        for k in range(NL):
            # ---- per-outer weights
            w_so_sb = load_w(wout, "w_so", d_w_so[:], 4 * SW, bf16)
            b_so_sb = load_w(wout, "b_so", d_b_so[:], SW, f32)
            w_e1_sb = load_w(wout, "w_e1", d_w_e1[:, k * 4 * SW : (k + 1) * 4 * SW], 4 * SW, bf16)
            b_e1_sb = load_w(wout, "b_e1", d_b_e1[:, k * SW : (k + 1) * SW], SW, f32)
            w_e2_sb = load_w(wout, "w_e2", d_w_e2[:, k * 4 * 128 : (k + 1) * 4 * 128], 4 * 128, bf16)
            b_e2_sb = load_w(wout, "b_e2", d_b_e2[:, k * 128 : (k + 1) * 128], 128, f32)
            ln_g_sb = load_w(wout, "ln_g", d_ln_g[:, k * HID : (k + 1) * HID], HID, f32)
            ln_b_sb = load_w(wout, "ln_b", d_ln_b[:, k * HID : (k + 1) * HID], HID, f32)
            w_b1_sb = load_w(wout, "w_b1", d_w_b1[:, k * 4 * HID : (k + 1) * 4 * HID], 4 * HID, bf16)
            b_b1_sb = load_w(wout, "b_b1", d_b_b1[:, k * HID : (k + 1) * HID], HID, f32)
            w_b2_sb = load_w(wout, "w_b2", d_w_b2[:, k * 2 * HID : (k + 1) * 2 * HID], 2 * HID, bf16)
            b_b2_sb = load_w(wout, "b_b2", d_b_b2[:, k * HID : (k + 1) * HID], HID, f32)

            # ---- z0 = gelu(h @ w_si + b_si), write to LOC
            loc_prev = dram_tile("loc", [NLOC, SW], bf16)
            for t in range(NT):
                nr = rows_of(t)
                hT = work.tile([128, 2 * 128], bf16, tag="hT", name="hT")
                transpose_into(hT, h_t[t][:], 2, True)
                zp = psum.tile([128, SW], f32, tag="mlp", name="zp")
                mm_acc(zp[:], hT, w_si_sb, 2, (0, SW), SW)
                zpre = work.tile([128, SW], f32, tag="tmpf", name="zpre")
                nc.vector.tensor_tensor(out=zpre[:], in0=zp[:], in1=b_si_sb[:], op=ALU.add)
                z0 = work.tile([128, SW], bf16, tag="znew", name="z0")
                nc.scalar.activation(out=z0[:], in_=zpre[:], func=ACT.Gelu)
                nc.sync.dma_start(out=loc_prev[t * 128 : t * 128 + nr, :], in_=z0[:nr, :])

            # ---- SAGE layers
            for i in range(cfg.NSAGE):
                agt = dram_tile("agt", [cfg.N, SW], bf16, shared=True)
                nc.gpsimd.collective_compute(
                    "AllGather",
                    ALU.bypass,
                    replica_groups=rg,
                    ins=[loc_prev.opt()],
                    outs=[agt.opt()],
                )
                w1_sb = load_w(wsage, "w1", d_w_s1[:, i * 8 * SW : (i + 1) * 8 * SW], 8 * SW, bf16)
                b1_sb = load_w(wsage, "b1", d_b_s1[:, i * SW : (i + 1) * SW], SW, f32)
                w2_sb = load_w(wsage, "w2", d_w_s2[:, i * 4 * SW : (i + 1) * 4 * SW], 4 * SW, bf16)
                b2_sb = load_w(wsage, "b2", d_b_s2[:, i * SW : (i + 1) * SW], SW, f32)
                last = i == cfg.NSAGE - 1
                loc_cur = None if last else dram_tile("loc", [NLOC, SW], bf16)
                if last:
                    locy = dram_tile("locy", [NLOC, HID], bf16)

                for t in range(NT):
                    nr = rows_of(t)
                    # aggregation
                    aps = psum.tile([128, SW], f32, tag="agg", name="aps")
                    emit_agg(agt[:], t, SW, aps[:])
                    m_sb = work.tile([128, SW], bf16, tag="msb", name="msb")
                    nc.vector.tensor_scalar(m_sb[:], aps[:], rdeg[:, t : t + 1], None, ALU.mult)
                    # self rows
                    z_sb = work.tile([128, SW], bf16, tag="zsb", name="zsb")
                    if nr < 128:
                        nc.gpsimd.memset(z_sb[:], 0.0)
                    nc.sync.dma_start(
                        out=z_sb[:nr, :], in_=loc_prev[t * 128 : t * 128 + nr, :]
                    )
                    # zc^T = [z | m]^T
                    zcT = work.tile([128, 8 * 128], bf16, tag="zcT", name="zcT")
                    transpose_into(zcT[:, : 4 * 128], z_sb[:], 4, False)
                    transpose_into(zcT[:, 4 * 128 : 8 * 128], m_sb[:], 4, False)
                    # MLP1
                    p1p = psum.tile([128, SW], f32, tag="mlp", name="p1p")
                    mm_acc(p1p[:], zcT, w1_sb, 8, (0, SW), SW)
                    p1pre = work.tile([128, SW], f32, tag="tmpf", name="p1pre")
                    nc.vector.tensor_tensor(out=p1pre[:], in0=p1p[:], in1=b1_sb[:], op=ALU.add)
                    p1 = work.tile([128, SW], bf16, tag="p1", name="p1")
                    nc.scalar.activation(out=p1[:], in_=p1pre[:], func=ACT.Gelu)
                    p1T = work.tile([128, 4 * 128], bf16, tag="p1T", name="p1T")
                    transpose_into(p1T, p1[:], 4, False)
                    # MLP2 + residual
                    p2p = psum.tile([128, SW], f32, tag="mlp", name="p2p")
                    mm_acc(p2p[:], p1T, w2_sb, 4, (0, SW), SW)
                    p2pre = work.tile([128, SW], f32, tag="tmpf", name="p2pre")
                    nc.vector.tensor_tensor(out=p2pre[:], in0=p2p[:], in1=b2_sb[:], op=ALU.add)
                    znew = work.tile([128, SW], bf16, tag="znew", name="znew")
                    nc.vector.tensor_tensor(out=znew[:], in0=p2pre[:], in1=z_sb[:], op=ALU.add)
                    if not last:
                        nc.sync.dma_start(
                            out=loc_cur[t * 128 : t * 128 + nr, :], in_=znew[:nr, :]
                        )
                        continue

                    # ---- fused: enc path -> rotation coefs; LN(h) -> y -> locy
                    z5T = work.tile([128, 4 * 128], bf16, tag="z5T", name="z5T")
                    transpose_into(z5T, znew[:], 4, False)
                    ep = psum.tile([128, SW], f32, tag="mlp", name="ep")
                    mm_acc(ep[:], z5T, w_so_sb, 4, (0, SW), SW)
                    enc = work.tile([128, SW], bf16, tag="enc", name="enc")
                    nc.vector.tensor_tensor(out=enc[:], in0=ep[:], in1=b_so_sb[:], op=ALU.add)
                    encT = work.tile([128, 4 * 128], bf16, tag="encT", name="encT")
                    transpose_into(encT, enc[:], 4, False)
                    gp = psum.tile([128, SW], f32, tag="mlp", name="gp")
                    mm_acc(gp[:], encT, w_e1_sb, 4, (0, SW), SW)
                    gpre = work.tile([128, SW], f32, tag="tmpf", name="gpre")
                    nc.vector.tensor_tensor(out=gpre[:], in0=gp[:], in1=b_e1_sb[:], op=ALU.add)
                    gact = work.tile([128, SW], bf16, tag="gact", name="gact")
                    nc.scalar.activation(out=gact[:], in_=gpre[:], func=ACT.Gelu)
                    gT = work.tile([128, 4 * 128], bf16, tag="gT", name="gT")
                    transpose_into(gT, gact[:], 4, False)
                    ap_ = psum.tile([128, 128], f32, tag="agg", name="ap_")
                    mm_acc(ap_[:], gT, w_e2_sb, 4, (0, 128), 128)
                    a_sb = work.tile([128, 128], f32, tag="a0", name="a_sb")
                    nc.vector.tensor_tensor(out=a_sb[:], in0=ap_[:], in1=b_e2_sb[:], op=ALU.add)
                    a2 = work.tile([128, 128], f32, tag="a1", name="a2")
                    nc.vector.tensor_tensor(out=a2[:], in0=a_sb[:], in1=a_sb[:], op=ALU.mult)
                    rinv = work.tile([128, 128], f32, tag="a2t", name="rinv")
                    nc.vector.tensor_scalar(rinv[:], a2[:], 1.0, None, ALU.add)
                    nc.vector.reciprocal(out=rinv[:], in_=rinv[:])
                    nc.vector.tensor_scalar(a2[:], a2[:], -1.0, None, ALU.add)
                    nc.vector.tensor_tensor(out=c_t[t][:], in0=a2[:], in1=rinv[:], op=ALU.mult)
                    nc.vector.tensor_scalar(a_sb[:], a_sb[:], 2.0, None, ALU.mult)
                    nc.vector.tensor_tensor(out=s_t[t][:], in0=a_sb[:], in1=rinv[:], op=ALU.mult)

                    # LN(h) -> hn; y = rot(hn)
                    hn = work.tile([128, HID], bf16, tag="hn", name="hn")
                    emit_ln(h_t[t][:], ln_g_sb[:], ln_b_sb[:], hn[:], HID)
                    hn_ev = hn[:, 0:HID:2]
                    hn_od = hn[:, 1:HID:2]
                    y = work.tile([128, HID], bf16, tag="y", name="y")
                    t0 = work.tile([128, 128], f32, tag="r0", name="t0")
                    t1 = work.tile([128, 128], f32, tag="r1", name="t1")
                    nc.vector.tensor_tensor(out=t0[:], in0=c_t[t][:], in1=hn_ev, op=ALU.mult)
                    nc.vector.tensor_tensor(out=t1[:], in0=s_t[t][:], in1=hn_od, op=ALU.mult)
                    nc.vector.tensor_tensor(out=y[:, 0:HID:2], in0=t0[:], in1=t1[:], op=ALU.add)
                    nc.vector.tensor_tensor(out=t0[:], in0=c_t[t][:], in1=hn_od, op=ALU.mult)
                    nc.vector.tensor_tensor(out=t1[:], in0=s_t[t][:], in1=hn_ev, op=ALU.mult)
                    nc.vector.tensor_tensor(
                        out=y[:, 1:HID:2], in0=t0[:], in1=t1[:], op=ALU.subtract
                    )
                    nc.sync.dma_start(out=locy[t * 128 : t * 128 + nr, :], in_=y[:nr, :])

                if not last:
                    loc_prev = loc_cur

            # ---- AllGather y, BDL message + MLP, h update
            agty = dram_tile("agty", [cfg.N, HID], bf16, shared=True)
            nc.gpsimd.collective_compute(
                "AllGather", ALU.bypass, replica_groups=rg, ins=[locy.opt()], outs=[agty.opt()]
            )
            for t in range(NT):
                yps = psum.tile([128, HID], f32, tag="agg", name="yps")
                emit_agg(agty[:], t, HID, yps[:])
                ga = work.tile([128, HID], f32, tag="ga", name="ga")
                nc.vector.tensor_scalar(ga[:], yps[:], rdeg[:, t : t + 1], None, ALU.mult)
                # hc = [LN(h) | msg]
                hc = work.tile([128, 2 * HID], bf16, tag="hc", name="hc")
                emit_ln(h_t[t][:], ln_g_sb[:], ln_b_sb[:], hc[:, :HID], HID)
                g_ev = ga[:, 0:HID:2]
                g_od = ga[:, 1:HID:2]
                t0 = work.tile([128, 128], f32, tag="r0", name="t0b")
                t1 = work.tile([128, 128], f32, tag="r1", name="t1b")
                nc.vector.tensor_tensor(out=t0[:], in0=c_t[t][:], in1=g_ev, op=ALU.mult)
                nc.vector.tensor_tensor(out=t1[:], in0=s_t[t][:], in1=g_od, op=ALU.mult)
                nc.vector.tensor_tensor(
                    out=hc[:, HID : 2 * HID : 2], in0=t0[:], in1=t1[:], op=ALU.subtract
                )
                nc.vector.tensor_tensor(out=t0[:], in0=s_t[t][:], in1=g_ev, op=ALU.mult)
                nc.vector.tensor_tensor(out=t1[:], in0=c_t[t][:], in1=g_od, op=ALU.mult)
                nc.vector.tensor_tensor(
                    out=hc[:, HID + 1 : 2 * HID : 2], in0=t0[:], in1=t1[:], op=ALU.add
                )
                hcT = work.tile([128, 4 * 128], bf16, tag="hcT", name="hcT")
                transpose_into(hcT, hc[:], 4, False)
                bp = psum.tile([128, HID], f32, tag="mlp", name="bp")
                mm_acc(bp[:], hcT, w_b1_sb, 4, (0, HID), HID)
                bpre = work.tile([128, HID], f32, tag="tmpf", name="bpre")
                nc.vector.tensor_tensor(out=bpre[:], in0=bp[:], in1=b_b1_sb[:], op=ALU.add)
                tb = work.tile([128, HID], bf16, tag="tb", name="tb")
                nc.scalar.activation(out=tb[:], in_=bpre[:], func=ACT.Gelu)
                tbT = work.tile([128, 2 * 128], bf16, tag="tbT", name="tbT")
                transpose_into(tbT, tb[:], 2, False)
                b2p = psum.tile([128, HID], f32, tag="mlp", name="b2p")
                mm_acc(b2p[:], tbT, w_b2_sb, 2, (0, HID), HID)
                dpre = work.tile([128, HID], f32, tag="tmpf", name="dpre")
                nc.vector.tensor_tensor(out=dpre[:], in0=b2p[:], in1=b_b2_sb[:], op=ALU.add)
                nc.vector.tensor_tensor(out=h_t[t][:], in0=h_t[t][:], in1=dpre[:], op=ALU.add)

        # ================= final LN + output =================
        oln_g_sb = load_w(wout, "oln_g", d_oln_g[:], HID, f32)
        oln_b_sb = load_w(wout, "oln_b", d_oln_b[:], HID, f32)
        w_o_sb = load_w(wout, "w_o", d_w_o[:], 2 * cfg.OUT, bf16)
        b_o_sb = load_w(wout, "b_o", d_b_o[:], cfg.OUT, f32)
        for t in range(NT):
            nr = rows_of(t)
            hnf = work.tile([128, HID], bf16, tag="hn", name="hnf")
            emit_ln(h_t[t][:], oln_g_sb[:], oln_b_sb[:], hnf[:], HID)
            hnfT = work.tile([128, 2 * 128], bf16, tag="tbT", name="hnfT")
            transpose_into(hnfT, hnf[:], 2, False)
            op_ = psum.tile([128, cfg.OUT], f32, tag="agg", name="op_")
            mm_acc(op_[:], hnfT, w_o_sb, 2, (0, cfg.OUT), cfg.OUT)
            ot = work.tile([128, cfg.OUT], f32, tag="ot", name="ot")
            nc.vector.tensor_tensor(out=ot[:], in0=op_[:], in1=b_o_sb[:], op=ALU.add)
            nc.sync.dma_start(out=d_out[t * 128 : t * 128 + nr, :], in_=ot[:nr, :])

        ctx.close()

    nc.compile()
    return nc


# ---------------------------------------------------------------- runner

_CACHE = {}


def _get_program(cfg: Cfg, CK: int):
    key = (cfg, CK)
    if key not in _CACHE:
        _CACHE[key] = build_program(cfg, CK)
    return _CACHE[key]


def run(inputs, cfg: Cfg = CFG, trace: bool = False):
    from concourse import bass_utils

    CK, in_maps = _prep_inputs(cfg, inputs)
    nc = _get_program(cfg, CK)
    res = bass_utils.run_bass_kernel_spmd(
        nc, in_maps, core_ids=list(range(cfg.NC)), trace=trace
    )
    out = np.concatenate([np.asarray(res.results[c]["out"]) for c in range(cfg.NC)], axis=0)
    return out, res


def kernel(**inputs):
    out, _ = run(inputs)
    return out



# revision 5
# speedup vs baseline: 1.3175x; 1.3175x over previous
"""Trainium2 Bass kernel for nn_BDLModel (gnn_message_passing).

Strategy (8 NeuronCores, SPMD):
  - Nodes sharded contiguously across cores (3750/core); within a core, nodes
    are assigned to 128-row dst tiles by balanced in-degree (LPT) to minimize
    the padded edge-chunk count CK; the output rows are inverse-permuted on
    the host.
  - Mean aggregation: the activation table is AllGathered to DRAM in fp8e4m3
    (values pre-scaled by 16; the 1/16 is folded into 1/deg), in 3 tile-group
    chunks issued as soon as each group's rows are written so the collective
    overlaps compute. Per dst-tile a dma_gather pulls all source rows
    ([128, CK, W] edge-major) and per 128-edge chunk a one-hot selection
    matrix S (iota vs per-edge dst-local ids) is the stationary matmul
    operand: psum[d, :] += S^T @ Zgathered; rdeg/16 applied on PSUM evacuation
    (Scalar engine).
  - Householder D=2 closed form: Q = [[c, s], [-s, c]], c=(a^2-1)/(1+a^2),
    s=2a/(1+a^2); only the 2::4 columns of enc_w2 are needed.
  - Self-z activations stay resident in SBUF; biases/residuals are preloaded
    into PSUM (Scalar/Vector) so matmul chains accumulate on top; PSUM
    evacuations and LayerNorm row-stats run on the Scalar engine.
"""

import math
import os
import sys
from dataclasses import dataclass

import numpy as np

for _p in ("/opt/trn_rl_repo", "/root/.axon_site/_ro/trn_rl_repo"):
    if os.path.isdir(_p) and _p not in sys.path:
        sys.path.insert(0, _p)

import ml_dtypes  # noqa: E402

BF16 = ml_dtypes.bfloat16

F8SCALE = 16.0


@dataclass(frozen=True)
class Cfg:
    N: int = 30000
    E: int = 480000
    HID: int = 256
    NB: int = 128
    D: int = 2
    NL: int = 2
    NSAGE: int = 5
    OUT: int = 5
    NC: int = 8
    EPS: float = 1e-5
    NGRP: int = 1

    @property
    def SW(self):
        return self.D * self.D * self.NB

    @property
    def NLOC(self):
        return self.N // self.NC

    @property
    def NT(self):
        return (self.NLOC + 127) // 128

    @property
    def GT(self):
        """Tile ranges per collective group."""
        per = (self.NT + self.NGRP - 1) // self.NGRP
        return [(g * per, min((g + 1) * per, self.NT)) for g in range(self.NGRP)]

    @property
    def GROWS(self):
        """(row_start, nrows) per group (local rows)."""
        out = []
        for t0, t1 in self.GT:
            r0 = t0 * 128
            r1 = min(t1 * 128, self.NLOC)
            out.append((r0, r1 - r0))
        return out


CFG = Cfg()


# ---------------------------------------------------------------- host prep


def _prep_rhs(w):
    """[K, M] -> [128, (K//128)*M] so slice kc -> [:, kc*M:(kc+1)*M] = W[kc]."""
    k, m = w.shape
    assert k % 128 == 0
    kc = k // 128
    return np.ascontiguousarray(
        w.reshape(kc, 128, m).transpose(1, 0, 2).reshape(128, kc * m)
    ).astype(BF16)


def _prep_bias(b):
    return np.ascontiguousarray(np.tile(np.asarray(b, np.float32).reshape(1, -1), (128, 1)))


def _balance_tiles(deg_local, nt):
    """LPT: assign nodes to nt tiles of <=128 nodes, balancing summed degree.
    Returns perm (tile-major node order)."""
    import heapq

    order = np.argsort(-deg_local, kind="stable")
    heap = [(0, t) for t in range(nt)]
    heapq.heapify(heap)
    counts = [0] * nt
    members = [[] for _ in range(nt)]
    for v in order:
        while True:
            load, t = heapq.heappop(heap)
            if counts[t] < 128:
                break
        members[t].append(v)
        counts[t] += 1
        if counts[t] < 128:
            heapq.heappush(heap, (load + int(deg_local[v]), t))
    return np.concatenate([np.asarray(m, np.int64) for m in members])


def _prep_edges(cfg: Cfg, edge_index):
    """Balanced tile assignment; edges partitioned by dst owner; per dst-tile
    padded chunk schedule with table rows in group-major collective layout."""
    src = np.asarray(edge_index[0], np.int64)
    dst = np.asarray(edge_index[1], np.int64)
    deg = np.bincount(dst, minlength=cfg.N).astype(np.float64)
    rdeg_full = (1.0 / np.maximum(deg, 1.0)).astype(np.float32) / F8SCALE

    NLOC, NT = cfg.NLOC, cfg.NT
    grows = cfg.GROWS
    # per-core balanced permutation: perm[c][p] = local node at tile-major pos p
    perms, poss = [], []
    for c in range(cfg.NC):
        dl = deg[c * NLOC : (c + 1) * NLOC]
        perm = _balance_tiles(dl, NT)
        pos = np.empty(NLOC, np.int64)
        pos[perm] = np.arange(NLOC)
        perms.append(perm)
        poss.append(pos)

    # table row of global node v (group-major AllGather layout)
    g_of_pos = np.empty(NLOC, np.int64)
    row_in_g = np.empty(NLOC, np.int64)
    pref = np.zeros(cfg.NGRP, np.int64)
    acc = 0
    for g, (r0, nr) in enumerate(grows):
        g_of_pos[r0 : r0 + nr] = g
        row_in_g[r0 : r0 + nr] = np.arange(nr)
        pref[g] = acc
        acc += nr

    def table_row(v):
        c = v // NLOC
        p = poss[c][v % NLOC]
        g = g_of_pos[p]
        return pref[g] * cfg.NC + c * (grows[g][1]) + row_in_g[p]

    # vectorized table_row over all src
    src_c = src // NLOC
    src_p = np.concatenate([poss[c][None, :] for c in range(cfg.NC)], axis=0)[
        src_c, src % NLOC
    ]
    src_g = g_of_pos[src_p]
    grows_nr = np.asarray([nr for _, nr in grows], np.int64)
    src_row = pref[src_g] * cfg.NC + src_c * grows_nr[src_g] + row_in_g[src_p]
    assert src_row.max() < 32768

    per_core = []
    ck_max = 1
    for c in range(cfg.NC):
        lo, hi = c * NLOC, (c + 1) * NLOC
        m = (dst >= lo) & (dst < hi)
        s_row = src_row[m]
        d_pos = poss[c][dst[m] - lo]
        order = np.argsort(d_pos, kind="stable")
        s_row, d_pos = s_row[order], d_pos[order]
        bounds = np.searchsorted(d_pos, np.arange(NT + 1) * 128)
        cnts = bounds[1:] - bounds[:-1]
        ck_c = max(1, int(math.ceil(cnts.max() / 128))) if len(s_row) else 1
        ck_max = max(ck_max, ck_c)
        per_core.append((s_row, d_pos, bounds))

    CK = ck_max
    outs = []
    for c in range(cfg.NC):
        s_row, d_pos, bounds = per_core[c]
        idx16 = np.zeros((128, NT * CK * 8), np.int16)
        ids = np.full((128, NT * CK), 255.0, BF16)
        for t in range(NT):
            b0, b1 = bounds[t], bounds[t + 1]
            n = b1 - b0
            if n == 0:
                continue
            i = np.arange(n)
            # gather order: unwrapped[i] = idx16[i%16, i//16] (replicated x8)
            col = t * CK * 8 + i // 16
            row = i % 16
            for g in range(8):
                idx16[row + 16 * g, col] = s_row[b0:b1]
            ids[i % 128, t * CK + i // 128] = (d_pos[b0:b1] - t * 128).astype(BF16)
        rdeg = np.ones((128, NT), np.float32) / F8SCALE
        rfull = rdeg_full[c * NLOC : (c + 1) * NLOC][perms[c]]
        for t in range(NT):
            r0 = t * 128
            nr = min(128, NLOC - r0)
            rdeg[:nr, t] = rfull[r0 : r0 + nr]
        outs.append(dict(idx16=idx16, ids=ids, rdeg=rdeg))
    return CK, outs, perms


def _prep_inputs(cfg: Cfg, inputs):
    """Build the per-core in_maps. Returns (CK, in_maps, perms)."""
    f32 = np.float32
    x = np.asarray(inputs["x"], f32)
    CK, edge_outs, perms = _prep_edges(cfg, np.asarray(inputs["edge_index"]))

    g = lambda k: np.asarray(inputs[k], f32)

    shared = {
        "w_in_r": _prep_rhs(g("w_in")),
        "b_in_bc": _prep_bias(g("b_in")),
        "w_si_r": _prep_rhs(g("se_in_w")),
        "b_si_bc": _prep_bias(g("se_in_b")),
        "w_s1_r": np.concatenate([_prep_rhs(g("sage_w1")[i]) for i in range(cfg.NSAGE)], axis=1),
        "b_s1_bc": np.concatenate([_prep_bias(g("sage_b1")[i]) for i in range(cfg.NSAGE)], axis=1),
        "w_s2_r": np.concatenate([_prep_rhs(g("sage_w2")[i]) for i in range(cfg.NSAGE)], axis=1),
        "b_s2_bc": np.concatenate([_prep_bias(g("sage_b2")[i]) for i in range(cfg.NSAGE)], axis=1),
        "w_so_r": _prep_rhs(g("se_out_w")),
        "b_so_bc": _prep_bias(g("se_out_b")),
        "w_e1_r": np.concatenate([_prep_rhs(g("enc_w1")[k]) for k in range(cfg.NL)], axis=1),
        "b_e1_bc": np.concatenate([_prep_bias(g("enc_b1")[k]) for k in range(cfg.NL)], axis=1),
        "w_e2_r": np.concatenate(
            [_prep_rhs(np.ascontiguousarray(g("enc_w2")[k][:, 2::4])) for k in range(cfg.NL)], axis=1
        ),
        "b_e2_bc": np.concatenate([_prep_bias(g("enc_b2")[k][2::4]) for k in range(cfg.NL)], axis=1),
        "ln_g_bc": np.concatenate([_prep_bias(g("ln_g")[k]) for k in range(cfg.NL)], axis=1),
        "ln_b_bc": np.concatenate([_prep_bias(g("ln_b")[k]) for k in range(cfg.NL)], axis=1),
        "w_b1_r": np.concatenate([_prep_rhs(g("bdl_w1")[k]) for k in range(cfg.NL)], axis=1),
        "b_b1_bc": np.concatenate([_prep_bias(g("bdl_b1")[k]) for k in range(cfg.NL)], axis=1),
        "w_b2_r": np.concatenate([_prep_rhs(g("bdl_w2")[k]) for k in range(cfg.NL)], axis=1),
        "b_b2_bc": np.concatenate([_prep_bias(g("bdl_b2")[k]) for k in range(cfg.NL)], axis=1),
        "oln_g_bc": _prep_bias(g("out_ln_g")),
        "oln_b_bc": _prep_bias(g("out_ln_b")),
        "w_o_r": _prep_rhs(g("w_out")),
        "b_o_bc": _prep_bias(g("b_out")),
        "ident_b": np.eye(128, dtype=BF16),
        "iota_f": np.tile(np.arange(128), (128, 1)).astype(BF16),
    }

    in_maps = []
    for c in range(cfg.NC):
        m = dict(shared)
        m["x_c"] = np.ascontiguousarray(x[c * cfg.NLOC : (c + 1) * cfg.NLOC][perms[c]])
        m["idx16"] = edge_outs[c]["idx16"]
        m["ids_f"] = edge_outs[c]["ids"]
        m["rdeg"] = edge_outs[c]["rdeg"]
        in_maps.append(m)
    return CK, in_maps, perms


# ---------------------------------------------------------------- builder


def build_program(cfg: Cfg, CK: int):
    from concourse import bacc, mybir
    import concourse.tile as tile

    f32 = mybir.dt.float32
    bf16 = mybir.dt.bfloat16
    fp8 = mybir.dt.float8e4
    i16 = mybir.dt.int16
    ALU = mybir.AluOpType
    AX = mybir.AxisListType
    ACT = mybir.ActivationFunctionType

    NT, NLOC, HID, SW = cfg.NT, cfg.NLOC, cfg.HID, cfg.SW
    GT, GROWS = cfg.GT, cfg.GROWS

    nc = bacc.Bacc(
        "TRN2",
        target_bir_lowering=False,
        debug=False,
        enable_asserts=False,
        num_devices=cfg.NC,
        num_swdge_queues=2,
    )
    rg = [list(range(cfg.NC))]

    # ---- external I/O
    d_x = nc.dram_tensor("x_c", [NLOC, HID], f32, kind="ExternalInput").ap()
    d_idx = nc.dram_tensor("idx16", [128, NT * CK * 8], i16, kind="ExternalInput").ap()
    d_ids = nc.dram_tensor("ids_f", [128, NT * CK], bf16, kind="ExternalInput").ap()
    d_rdeg = nc.dram_tensor("rdeg", [128, NT], f32, kind="ExternalInput").ap()

    def din(name, shape, dt):
        return nc.dram_tensor(name, shape, dt, kind="ExternalInput").ap()

    NS, NL = cfg.NSAGE, cfg.NL
    d_w_in = din("w_in_r", [128, 2 * HID], bf16)
    d_b_in = din("b_in_bc", [128, HID], f32)
    d_w_si = din("w_si_r", [128, 2 * SW], bf16)
    d_b_si = din("b_si_bc", [128, SW], f32)
    d_w_s1 = din("w_s1_r", [128, NS * 8 * SW], bf16)
    d_b_s1 = din("b_s1_bc", [128, NS * SW], f32)
    d_w_s2 = din("w_s2_r", [128, NS * 4 * SW], bf16)
    d_b_s2 = din("b_s2_bc", [128, NS * SW], f32)
    d_w_so = din("w_so_r", [128, 4 * SW], bf16)
    d_b_so = din("b_so_bc", [128, SW], f32)
    d_w_e1 = din("w_e1_r", [128, NL * 4 * SW], bf16)
    d_b_e1 = din("b_e1_bc", [128, NL * SW], f32)
    d_w_e2 = din("w_e2_r", [128, NL * 4 * 128], bf16)
    d_b_e2 = din("b_e2_bc", [128, NL * 128], f32)
    d_ln_g = din("ln_g_bc", [128, NL * HID], f32)
    d_ln_b = din("ln_b_bc", [128, NL * HID], f32)
    d_w_b1 = din("w_b1_r", [128, NL * 4 * HID], bf16)
    d_b_b1 = din("b_b1_bc", [128, NL * HID], f32)
    d_w_b2 = din("w_b2_r", [128, NL * 2 * HID], bf16)
    d_b_b2 = din("b_b2_bc", [128, NL * HID], f32)
    d_oln_g = din("oln_g_bc", [128, HID], f32)
    d_oln_b = din("oln_b_bc", [128, HID], f32)
    d_w_o = din("w_o_r", [128, 2 * cfg.OUT], bf16)
    d_b_o = din("b_o_bc", [128, cfg.OUT], f32)
    d_identb = din("ident_b", [128, 128], bf16)
    d_iota = din("iota_f", [128, 128], bf16)

    d_out = nc.dram_tensor("out", [NLOC, cfg.OUT], f32, kind="ExternalOutput").ap()

    with tile.TileContext(nc) as tc:
        from contextlib import ExitStack

        ctx = ExitStack()
        pers = ctx.enter_context(tc.tile_pool(name="pers", bufs=1))
        wout = ctx.enter_context(tc.tile_pool(name="wout", bufs=1))
        wsage = ctx.enter_context(tc.tile_pool(name="wsage", bufs=1))
        work = ctx.enter_context(tc.tile_pool(name="work", bufs=2))
        small = ctx.enter_context(tc.tile_pool(name="small", bufs=2))
        spool = ctx.enter_context(tc.tile_pool(name="spool", bufs=2))
        zgp = ctx.enter_context(tc.tile_pool(name="zgp", bufs=2))
        dram = ctx.enter_context(tc.tile_pool(name="dram", bufs=2, space="DRAM"))
        psum = ctx.enter_context(tc.tile_pool(name="psum", bufs=2, space="PSUM"))

        # ---- persistent SBUF residents
        identb = pers.tile([128, 128], bf16, name="identb")
        iota = pers.tile([128, 128], bf16, name="iota")
        rdeg = pers.tile([128, NT], f32, name="rdegs")
        ids = pers.tile([128, NT * CK], bf16, name="idss")
        idx = pers.tile([128, NT * CK * 8], i16, name="idxs")
        nc.sync.dma_start(out=identb[:], in_=d_identb[:])
        nc.sync.dma_start(out=iota[:], in_=d_iota[:])
        nc.sync.dma_start(out=rdeg[:], in_=d_rdeg[:])
        nc.sync.dma_start(out=ids[:], in_=d_ids[:])
        nc.sync.dma_start(out=idx[:], in_=d_idx[:])

        h_t = [pers.tile([128, HID], f32, name=f"h{t}") for t in range(NT)]
        z_t = [pers.tile([128, SW], bf16, name=f"z{t}") for t in range(NT)]
        c_t = [pers.tile([128, 128], bf16, name=f"rc{t}") for t in range(NT)]
        s_t = [pers.tile([128, 128], bf16, name=f"rs{t}") for t in range(NT)]

        def load_w(pool, name, src, cols, dt):
            t = pool.tile([128, cols], dt, tag=name, name=name)
            nc.sync.dma_start(out=t[:], in_=src)
            return t

        def dram_tile(name, shape, dt, shared=False):
            return dram.tile(
                shape, dt, tag=name, name=name, addr_space="Shared" if shared else "Local"
            )

        def rows_of(t):
            return min(128, NLOC - t * 128)

        # ---------- helpers ----------
        def transpose_into(dst, src_ap, nchunks):
            """dst[:, kc*128:(kc+1)*128] = src[:, kc*128:(kc+1)*128]^T (bf16)."""
            for kc in range(nchunks):
                tp = psum.tile([128, 128], bf16, tag="tr", name="trb")
                nc.tensor.transpose(
                    tp[:], src_ap[:, kc * 128 : (kc + 1) * 128], identb[:]
                )
                nc.scalar.copy(out=dst[:, kc * 128 : (kc + 1) * 128], in_=tp[:])

        def mm_acc(ps_ap, lhsT_tile, rhs_tile, kcs, m, rhs_block, preloaded=False):
            """ps (+)= sum_kc lhsT[:, kc]^T @ rhs[:, kc-block] (node-major out)."""
            for kc in range(kcs):
                nc.tensor.matmul(
                    ps_ap,
                    lhsT=lhsT_tile[:, kc * 128 : (kc + 1) * 128],
                    rhs=rhs_tile[:, kc * rhs_block + m[0] : kc * rhs_block + m[1]],
                    start=(kc == 0 and not preloaded),
                    stop=(kc == kcs - 1),
                    skip_group_check=preloaded,
                )

        def emit_ln(h_ap, g_bc_ap, b_bc_ap, out_ap, w):
            """LayerNorm with row stats on the Scalar engine."""
            junk = work.tile([128, w], f32, tag="lnjunk", name="lnjunk")
            s1 = small.tile([128, 1], f32, tag="ln1", name="ln1")
            nc.scalar.activation(out=junk[:], in_=h_ap, func=ACT.Identity, accum_out=s1[:])
            nm = small.tile([128, 1], f32, tag="ln2", name="ln2")
            nc.vector.tensor_scalar(nm[:], s1[:], -1.0 / w, None, ALU.mult)
            junk2 = work.tile([128, w], f32, tag="lnjunk2", name="lnjunk2")
            v = small.tile([128, 1], f32, tag="ln3", name="ln3")
            nc.scalar.activation(
                out=junk2[:], in_=h_ap, func=ACT.Square, bias=nm[:], accum_out=v[:]
            )
            vm = small.tile([128, 1], f32, tag="ln4", name="ln4")
            nc.vector.tensor_scalar(vm[:], v[:], 1.0 / w, cfg.EPS, ALU.mult, ALU.add)
            r = small.tile([128, 1], f32, tag="ln5", name="ln5")
            nc.vector.reciprocal(out=r[:], in_=vm[:])
            rs = small.tile([128, 1], f32, tag="ln6", name="ln6")
            nc.scalar.sqrt(out=rs[:], in_=r[:])
            nmrs = small.tile([128, 1], f32, tag="ln7", name="ln7")
            nc.vector.tensor_tensor(out=nmrs[:], in0=nm[:], in1=rs[:], op=ALU.mult)
            hn0 = work.tile([128, w], bf16, tag="lnhn0", name="lnhn0")
            nc.scalar.activation(
                out=hn0[:], in_=h_ap, func=ACT.Identity, bias=nmrs[:], scale=rs[:]
            )
            hn1 = work.tile([128, w], bf16, tag="lnhn1", name="lnhn1")
            nc.vector.tensor_tensor(out=hn1[:], in0=hn0[:], in1=g_bc_ap, op=ALU.mult)
            nc.vector.tensor_tensor(out=out_ap, in0=hn1[:], in1=b_bc_ap, op=ALU.add)

        def emit_agg(table_ap, t, width, ps_ap):
            """Gather + one-hot matmul segment sum for dst tile t into psum."""
            zg = zgp.tile([128, CK, width], fp8, tag="zg", name="zg")
            cka = (CK + 1) // 2
            for qi, (c0, c1) in enumerate(((0, cka), (cka, CK))):
                if c1 > c0:
                    nc.gpsimd.dma_gather(
                        out_ap=zg[:, c0:c1, :],
                        in_ap=table_ap,
                        idxs_ap=idx[:, t * CK * 8 + c0 * 8 : t * CK * 8 + c1 * 8],
                        num_idxs=(c1 - c0) * 128,
                        num_idxs_reg=(c1 - c0) * 128,
                        elem_size=width,
                        single_packet=False,
                        queue_num=qi,
                    )
            Sall = spool.tile([128, CK * 128], bf16, tag="S", name="S")
            iota_bc = iota[:].rearrange("p (o f) -> p o f", o=1).to_broadcast([128, CK, 128])
            ids_bc = (
                ids[:, t * CK : (t + 1) * CK]
                .rearrange("p (c o) -> p c o", o=1)
                .to_broadcast([128, CK, 128])
            )
            nc.vector.tensor_tensor(out=Sall[:], in0=iota_bc, in1=ids_bc, op=ALU.is_equal)
            for c in range(CK):
                nc.tensor.matmul(
                    ps_ap,
                    lhsT=Sall[:, c * 128 : (c + 1) * 128],
                    rhs=zg[:, c, :],
                    start=(c == 0),
                    stop=(c == CK - 1),
                )

        def fire_groups(t, loc_ap, tab_ap, width, done_groups):
            """Issue group AllGathers whose tile range completed at tile t."""
            for g, (t0, t1) in enumerate(GT):
                if t == t1 - 1 and g not in done_groups:
                    done_groups.add(g)
                    r0, nr = GROWS[g]
                    off = sum(GROWS[gg][1] for gg in range(g)) * cfg.NC
                    nc.gpsimd.collective_compute(
                        "AllGather",
                        ALU.bypass,
                        replica_groups=rg,
                        ins=[loc_ap[r0 : r0 + nr, :]],
                        outs=[tab_ap[off : off + nr * cfg.NC, :]],
                    )

        def emit_z0(t, w_si_sb, b_si_sb, loc8_ap, tab8_ap, done, hb16):
            """z0 = gelu(h@w_si+b_si) -> z_t[t] (bf16) + fp8 table row write."""
            nr = rows_of(t)
            hT = work.tile([128, 2 * 128], bf16, tag="hT", name="hT")
            transpose_into(hT, hb16[:], 2)
            zp = psum.tile([128, SW], f32, tag="mlp", name="zp")
            nc.scalar.copy(out=zp[:], in_=b_si_sb[:])
            mm_acc(zp[:], hT, w_si_sb, 2, (0, SW), SW, preloaded=True)
            nc.scalar.activation(out=z_t[t][:], in_=zp[:], func=ACT.Gelu)
            z8 = work.tile([128, SW], fp8, tag="z8", name="z8")
            nc.vector.tensor_scalar(z8[:], z_t[t][:], F8SCALE, None, ALU.mult)
            nc.sync.dma_start(out=loc8_ap[t * 128 : t * 128 + nr, :], in_=z8[:nr, :])
            fire_groups(t, loc8_ap, tab8_ap, SW, done)

        # ================= phase 0: h0 = gelu(x @ w_in + b_in), z0 =================
        w_in_sb = load_w(wout, "w_in", d_w_in[:], 2 * HID, bf16)
        b_in_sb = load_w(wout, "b_in", d_b_in[:], HID, f32)
        w_si_sb = load_w(wout, "w_si", d_w_si[:], 2 * SW, bf16)
        b_si_sb = load_w(wout, "b_si", d_b_si[:], SW, f32)

        loc8 = dram_tile("loc8", [NLOC, SW], fp8)
        tab8 = dram_tile("tab8", [cfg.N, SW], fp8, shared=True)
        done0 = set()
        for t in range(NT):
            nr = rows_of(t)
            xt = work.tile([128, HID], f32, tag="xt", name="xt")
            if nr < 128:
                nc.gpsimd.memset(xt[:], 0.0)
            nc.sync.dma_start(out=xt[:nr, :], in_=d_x[t * 128 : t * 128 + nr, :])
            xb = work.tile([128, HID], bf16, tag="hb16", name="xb")
            nc.vector.tensor_copy(out=xb[:], in_=xt[:])
            xT = work.tile([128, 2 * 128], bf16, tag="xT", name="xT")
            transpose_into(xT, xb[:], 2)
            hp = psum.tile([128, HID], f32, tag="mlp", name="hp")
            nc.scalar.copy(out=hp[:], in_=b_in_sb[:])
            mm_acc(hp[:], xT, w_in_sb, 2, (0, HID), HID, preloaded=True)
            nc.scalar.activation(out=h_t[t][:], in_=hp[:], func=ACT.Gelu)
            hb16 = work.tile([128, HID], bf16, tag="hb16", name="hb16")
            nc.vector.tensor_copy(out=hb16[:], in_=h_t[t][:])
            emit_z0(t, w_si_sb, b_si_sb, loc8, tab8, done0, hb16)

        # ================= outer layers =================
        for k in range(NL):
            # ---- per-outer weights
            w_so_sb = load_w(wout, "w_so", d_w_so[:], 4 * SW, bf16)
            b_so_sb = load_w(wout, "b_so", d_b_so[:], SW, f32)
            w_e1_sb = load_w(wout, "w_e1", d_w_e1[:, k * 4 * SW : (k + 1) * 4 * SW], 4 * SW, bf16)
            b_e1_sb = load_w(wout, "b_e1", d_b_e1[:, k * SW : (k + 1) * SW], SW, f32)
            w_e2_sb = load_w(wout, "w_e2", d_w_e2[:, k * 4 * 128 : (k + 1) * 4 * 128], 4 * 128, bf16)
            b_e2_sb = load_w(wout, "b_e2", d_b_e2[:, k * 128 : (k + 1) * 128], 128, f32)
            ln_g_sb = load_w(wout, "ln_g", d_ln_g[:, k * HID : (k + 1) * HID], HID, f32)
            ln_b_sb = load_w(wout, "ln_b", d_ln_b[:, k * HID : (k + 1) * HID], HID, f32)
            w_b1_sb = load_w(wout, "w_b1", d_w_b1[:, k * 4 * HID : (k + 1) * 4 * HID], 4 * HID, bf16)
            b_b1_sb = load_w(wout, "b_b1", d_b_b1[:, k * HID : (k + 1) * HID], HID, f32)
            w_b2_sb = load_w(wout, "w_b2", d_w_b2[:, k * 2 * HID : (k + 1) * 2 * HID], 2 * HID, bf16)
            b_b2_sb = load_w(wout, "b_b2", d_b_b2[:, k * HID : (k + 1) * HID], HID, f32)

            tab_prev = tab8
            locy8 = dram_tile("locy8", [NLOC, HID], fp8)
            ytab8 = dram_tile("ytab8", [cfg.N, HID], fp8, shared=True)
            doney = set()

            # ---- SAGE layers
            for i in range(cfg.NSAGE):
                w1_sb = load_w(wsage, "w1", d_w_s1[:, i * 8 * SW : (i + 1) * 8 * SW], 8 * SW, bf16)
                b1_sb = load_w(wsage, "b1", d_b_s1[:, i * SW : (i + 1) * SW], SW, f32)
                w2_sb = load_w(wsage, "w2", d_w_s2[:, i * 4 * SW : (i + 1) * 4 * SW], 4 * SW, bf16)
                b2_sb = load_w(wsage, "b2", d_b_s2[:, i * SW : (i + 1) * SW], SW, f32)
                last = i == cfg.NSAGE - 1
                if not last:
                    loc8_cur = dram_tile("loc8", [NLOC, SW], fp8)
                    tab8_cur = dram_tile("tab8", [cfg.N, SW], fp8, shared=True)
                    done_cur = set()

                for t in range(NT):
                    nr = rows_of(t)
                    # aggregation from previous table
                    aps = psum.tile([128, SW], f32, tag="agg", name="aps")
                    emit_agg(tab_prev[:], t, SW, aps[:])
                    m_sb = work.tile([128, SW], bf16, tag="msb", name="msb")
                    nc.scalar.mul(out=m_sb[:], in_=aps[:], mul=rdeg[:, t : t + 1])
                    # zc^T = [z | m]^T
                    zcT = work.tile([128, 8 * 128], bf16, tag="zcT", name="zcT")
                    transpose_into(zcT[:, : 4 * 128], z_t[t][:], 4)
                    transpose_into(zcT[:, 4 * 128 : 8 * 128], m_sb[:], 4)
                    # MLP1
                    p1p = psum.tile([128, SW], f32, tag="mlp", name="p1p")
                    nc.scalar.copy(out=p1p[:], in_=b1_sb[:])
                    mm_acc(p1p[:], zcT, w1_sb, 8, (0, SW), SW, preloaded=True)
                    p1 = work.tile([128, SW], bf16, tag="p1", name="p1")
                    nc.scalar.activation(out=p1[:], in_=p1p[:], func=ACT.Gelu)
                    p1T = work.tile([128, 4 * 128], bf16, tag="p1T", name="p1T")
                    transpose_into(p1T, p1[:], 4)
                    # MLP2 + residual (z + b2 preloaded into psum)
                    p2p = psum.tile([128, SW], f32, tag="mlp", name="p2p")
                    nc.vector.tensor_tensor(out=p2p[:], in0=z_t[t][:], in1=b2_sb[:], op=ALU.add)
                    mm_acc(p2p[:], p1T, w2_sb, 4, (0, SW), SW, preloaded=True)
                    nc.scalar.copy(out=z_t[t][:], in_=p2p[:])
                    if not last:
                        z8 = work.tile([128, SW], fp8, tag="z8", name="z8")
                        nc.vector.tensor_scalar(z8[:], p2p[:], F8SCALE, None, ALU.mult)
                        nc.sync.dma_start(
                            out=loc8_cur[t * 128 : t * 128 + nr, :], in_=z8[:nr, :]
                        )
                        fire_groups(t, loc8_cur, tab8_cur, SW, done_cur)
                        continue

                    # ---- fused: enc path -> rotation coefs; LN(h) -> y -> locy8
                    z5T = work.tile([128, 4 * 128], bf16, tag="z5T", name="z5T")
                    transpose_into(z5T, z_t[t][:], 4)
                    ep = psum.tile([128, SW], f32, tag="mlp", name="ep")
                    nc.scalar.copy(out=ep[:], in_=b_so_sb[:])
                    mm_acc(ep[:], z5T, w_so_sb, 4, (0, SW), SW, preloaded=True)
                    enc = work.tile([128, SW], bf16, tag="enc", name="enc")
                    nc.scalar.copy(out=enc[:], in_=ep[:])
                    encT = work.tile([128, 4 * 128], bf16, tag="encT", name="encT")
                    transpose_into(encT, enc[:], 4)
                    gp = psum.tile([128, SW], f32, tag="mlp", name="gp")
                    nc.scalar.copy(out=gp[:], in_=b_e1_sb[:])
                    mm_acc(gp[:], encT, w_e1_sb, 4, (0, SW), SW, preloaded=True)
                    gact = work.tile([128, SW], bf16, tag="gact", name="gact")
                    nc.scalar.activation(out=gact[:], in_=gp[:], func=ACT.Gelu)
                    gT = work.tile([128, 4 * 128], bf16, tag="gT", name="gT")
                    transpose_into(gT, gact[:], 4)
                    ap_ = psum.tile([128, 128], f32, tag="agg", name="ap_")
                    nc.scalar.copy(out=ap_[:], in_=b_e2_sb[:])
                    mm_acc(ap_[:], gT, w_e2_sb, 4, (0, 128), 128, preloaded=True)
                    a_sb = work.tile([128, 128], f32, tag="a0", name="a_sb")
                    nc.scalar.copy(out=a_sb[:], in_=ap_[:])
                    a2 = work.tile([128, 128], f32, tag="a1", name="a2")
                    nc.vector.tensor_tensor(out=a2[:], in0=a_sb[:], in1=a_sb[:], op=ALU.mult)
                    rinv = work.tile([128, 128], f32, tag="a2t", name="rinv")
                    nc.vector.tensor_scalar(rinv[:], a2[:], 1.0, None, ALU.add)
                    nc.vector.reciprocal(out=rinv[:], in_=rinv[:])
                    nc.vector.tensor_scalar(a2[:], a2[:], -1.0, None, ALU.add)
                    nc.vector.tensor_tensor(out=c_t[t][:], in0=a2[:], in1=rinv[:], op=ALU.mult)
                    nc.vector.tensor_scalar(a_sb[:], a_sb[:], 2.0, None, ALU.mult)
                    nc.vector.tensor_tensor(out=s_t[t][:], in0=a_sb[:], in1=rinv[:], op=ALU.mult)

                    # LN(h) -> hn; y = rot(hn); y8 = y*16 fp8
                    hn = work.tile([128, HID], bf16, tag="hn", name="hn")
                    emit_ln(h_t[t][:], ln_g_sb[:], ln_b_sb[:], hn[:], HID)
                    hn_ev = hn[:, 0:HID:2]
                    hn_od = hn[:, 1:HID:2]
                    y = work.tile([128, HID], bf16, tag="y", name="y")
                    t0 = work.tile([128, 128], bf16, tag="r0", name="t0")
                    t1 = work.tile([128, 128], bf16, tag="r1", name="t1")
                    nc.vector.tensor_tensor(out=t0[:], in0=c_t[t][:], in1=hn_ev, op=ALU.mult)
                    nc.vector.tensor_tensor(out=t1[:], in0=s_t[t][:], in1=hn_od, op=ALU.mult)
                    nc.vector.tensor_tensor(out=y[:, 0:HID:2], in0=t0[:], in1=t1[:], op=ALU.add)
                    nc.vector.tensor_tensor(out=t0[:], in0=c_t[t][:], in1=hn_od, op=ALU.mult)
                    nc.vector.tensor_tensor(out=t1[:], in0=s_t[t][:], in1=hn_ev, op=ALU.mult)
                    nc.vector.tensor_tensor(
                        out=y[:, 1:HID:2], in0=t0[:], in1=t1[:], op=ALU.subtract
                    )
                    y8 = work.tile([128, HID], fp8, tag="y8", name="y8")
                    nc.vector.tensor_scalar(y8[:], y[:], F8SCALE, None, ALU.mult)
                    nc.sync.dma_start(
                        out=locy8[t * 128 : t * 128 + nr, :], in_=y8[:nr, :]
                    )
                    fire_groups(t, locy8, ytab8, HID, doney)

                if not last:
                    tab_prev = tab8_cur
                    loc8 = loc8_cur

            # ---- BDL message + MLP, h update (+ fused z0 of next layer / output)
            if k + 1 < NL:
                loc8_nxt = dram_tile("loc8", [NLOC, SW], fp8)
                tab8_nxt = dram_tile("tab8", [cfg.N, SW], fp8, shared=True)
                done_nxt = set()
            else:
                oln_g_sb = load_w(wout, "oln_g", d_oln_g[:], HID, f32)
                oln_b_sb = load_w(wout, "oln_b", d_oln_b[:], HID, f32)
                w_o_sb = load_w(wout, "w_o", d_w_o[:], 2 * cfg.OUT, bf16)
                b_o_sb = load_w(wout, "b_o", d_b_o[:], cfg.OUT, f32)

            for t in range(NT):
                nr = rows_of(t)
                yps = psum.tile([128, HID], f32, tag="agg", name="yps")
                emit_agg(ytab8[:], t, HID, yps[:])
                ga = work.tile([128, HID], bf16, tag="ga", name="ga")
                nc.scalar.mul(out=ga[:], in_=yps[:], mul=rdeg[:, t : t + 1])
                # hn and msg (rotated back aggregate)
                hn = work.tile([128, HID], bf16, tag="hn", name="hnb")
                emit_ln(h_t[t][:], ln_g_sb[:], ln_b_sb[:], hn[:], HID)
                g_ev = ga[:, 0:HID:2]
                g_od = ga[:, 1:HID:2]
                msg = work.tile([128, HID], bf16, tag="msg", name="msg")
                t0 = work.tile([128, 128], bf16, tag="r0", name="t0b")
                t1 = work.tile([128, 128], bf16, tag="r1", name="t1b")
                nc.vector.tensor_tensor(out=t0[:], in0=c_t[t][:], in1=g_ev, op=ALU.mult)
                nc.vector.tensor_tensor(out=t1[:], in0=s_t[t][:], in1=g_od, op=ALU.mult)
                nc.vector.tensor_tensor(
                    out=msg[:, 0:HID:2], in0=t0[:], in1=t1[:], op=ALU.subtract
                )
                nc.vector.tensor_tensor(out=t0[:], in0=s_t[t][:], in1=g_ev, op=ALU.mult)
                nc.vector.tensor_tensor(out=t1[:], in0=c_t[t][:], in1=g_od, op=ALU.mult)
                nc.vector.tensor_tensor(
                    out=msg[:, 1:HID:2], in0=t0[:], in1=t1[:], op=ALU.add
                )
                hcT = work.tile([128, 4 * 128], bf16, tag="hcT", name="hcT")
                transpose_into(hcT[:, : 2 * 128], hn[:], 2)
                transpose_into(hcT[:, 2 * 128 : 4 * 128], msg[:], 2)
                bp = psum.tile([128, HID], f32, tag="mlp", name="bp")
                nc.scalar.copy(out=bp[:], in_=b_b1_sb[:])
                mm_acc(bp[:], hcT, w_b1_sb, 4, (0, HID), HID, preloaded=True)
                tb = work.tile([128, HID], bf16, tag="tb", name="tb")
                nc.scalar.activation(out=tb[:], in_=bp[:], func=ACT.Gelu)
                tbT = work.tile([128, 2 * 128], bf16, tag="tbT", name="tbT")
                transpose_into(tbT, tb[:], 2)
                b2p = psum.tile([128, HID], f32, tag="mlp", name="b2p")
                nc.vector.tensor_tensor(out=b2p[:], in0=h_t[t][:], in1=b_b2_sb[:], op=ALU.add)
                mm_acc(b2p[:], tbT, w_b2_sb, 2, (0, HID), HID, preloaded=True)
                nc.scalar.copy(out=h_t[t][:], in_=b2p[:])

                if k + 1 < NL:
                    hb16 = work.tile([128, HID], bf16, tag="hb16", name="hb16b")
                    nc.vector.tensor_copy(out=hb16[:], in_=h_t[t][:])
                    emit_z0(t, w_si_sb, b_si_sb, loc8_nxt, tab8_nxt, done_nxt, hb16)
                else:
                    hnf = work.tile([128, HID], bf16, tag="hn", name="hnf")
                    emit_ln(h_t[t][:], oln_g_sb[:], oln_b_sb[:], hnf[:], HID)
                    hnfT = work.tile([128, 2 * 128], bf16, tag="tbT", name="hnfT")
                    transpose_into(hnfT, hnf[:], 2)
                    op_ = psum.tile([128, cfg.OUT], f32, tag="mlp", name="op_")
                    nc.scalar.copy(out=op_[:], in_=b_o_sb[:])
                    mm_acc(op_[:], hnfT, w_o_sb, 2, (0, cfg.OUT), cfg.OUT, preloaded=True)
                    ot = work.tile([128, cfg.OUT], f32, tag="ot", name="ot")
                    nc.scalar.copy(out=ot[:], in_=op_[:])
                    nc.sync.dma_start(out=d_out[t * 128 : t * 128 + nr, :], in_=ot[:nr, :])

            if k + 1 < NL:
                tab8 = tab8_nxt
                loc8 = loc8_nxt

        ctx.close()

    nc.compile()
    return nc


# ---------------------------------------------------------------- runner

_CACHE = {}


def _get_program(cfg: Cfg, CK: int):
    key = (cfg, CK)
    if key not in _CACHE:
        _CACHE[key] = build_program(cfg, CK)
    return _CACHE[key]


def run(inputs, cfg: Cfg = CFG, trace: bool = False):
    from concourse import bass_utils

    CK, in_maps, perms = _prep_inputs(cfg, inputs)
    nc = _get_program(cfg, CK)
    res = bass_utils.run_bass_kernel_spmd(
        nc, in_maps, core_ids=list(range(cfg.NC)), trace=trace
    )
    out = np.empty((cfg.N, cfg.OUT), np.float32)
    for c in range(cfg.NC):
        out[c * cfg.NLOC + perms[c]] = np.asarray(res.results[c]["out"])
    return out, res


def kernel(**inputs):
    out, _ = run(inputs)
    return out


# revision 9
# speedup vs baseline: 1.3901x; 1.0551x over previous
"""Trainium2 Bass kernel for nn_BDLModel (gnn_message_passing).

Strategy (8 NeuronCores, SPMD):
  - Nodes sharded contiguously across cores (3750/core); within a core, nodes
    are assigned to 128-row dst tiles by balanced in-degree (LPT) to minimize
    the padded edge-chunk count CK; the output rows are inverse-permuted on
    the host.
  - Mean aggregation: the activation table is AllGathered to DRAM in fp8e4m3
    (values pre-scaled by 16; the 1/16 is folded into 1/deg), in 3 tile-group
    chunks issued as soon as each group's rows are written so the collective
    overlaps compute. Per dst-tile a dma_gather pulls all source rows
    ([128, CK, W] edge-major) and per 128-edge chunk a one-hot selection
    matrix S (iota vs per-edge dst-local ids) is the stationary matmul
    operand: psum[d, :] += S^T @ Zgathered; rdeg/16 applied on PSUM evacuation
    (Scalar engine).
  - Householder D=2 closed form: Q = [[c, s], [-s, c]], c=(a^2-1)/(1+a^2),
    s=2a/(1+a^2); only the 2::4 columns of enc_w2 are needed.
  - Self-z activations stay resident in SBUF; biases/residuals are preloaded
    into PSUM (Scalar/Vector) so matmul chains accumulate on top; PSUM
    evacuations and LayerNorm row-stats run on the Scalar engine.
"""

import math
import os
import sys
from dataclasses import dataclass

import numpy as np

for _p in ("/opt/trn_rl_repo", "/root/.axon_site/_ro/trn_rl_repo"):
    if os.path.isdir(_p) and _p not in sys.path:
        sys.path.insert(0, _p)

import ml_dtypes  # noqa: E402

BF16 = ml_dtypes.bfloat16

F8SCALE = 16.0


@dataclass(frozen=True)
class Cfg:
    N: int = 30000
    E: int = 480000
    HID: int = 256
    NB: int = 128
    D: int = 2
    NL: int = 2
    NSAGE: int = 5
    OUT: int = 5
    NC: int = 8
    EPS: float = 1e-5

    @property
    def SW(self):
        return self.D * self.D * self.NB

    @property
    def NLOC(self):
        return self.N // self.NC

    @property
    def NT(self):
        return (self.NLOC + 127) // 128

    @property
    def HLF(self):
        """Half-split row boundary for the two per-layer table collectives."""
        return self.NLOC // 2

    @property
    def HT(self):
        """First tile index whose rows complete the A half."""
        return (self.HLF + 127) // 128


CFG = Cfg()


# ---------------------------------------------------------------- host prep


def _prep_rhs(w):
    """[K, M] -> [128, (K//128)*M] so slice kc -> [:, kc*M:(kc+1)*M] = W[kc]."""
    k, m = w.shape
    assert k % 128 == 0
    kc = k // 128
    return np.ascontiguousarray(
        w.reshape(kc, 128, m).transpose(1, 0, 2).reshape(128, kc * m)
    ).astype(BF16)


def _prep_bias(b):
    return np.ascontiguousarray(np.tile(np.asarray(b, np.float32).reshape(1, -1), (128, 1)))


def _balance_tiles(deg_local, nt):
    """LPT: assign nodes to nt tiles of <=128 nodes, balancing summed degree.
    Returns perm (tile-major node order)."""
    import heapq

    order = np.argsort(-deg_local, kind="stable")
    heap = [(0, t) for t in range(nt)]
    heapq.heapify(heap)
    counts = [0] * nt
    members = [[] for _ in range(nt)]
    for v in order:
        while True:
            load, t = heapq.heappop(heap)
            if counts[t] < 128:
                break
        members[t].append(v)
        counts[t] += 1
        if counts[t] < 128:
            heapq.heappush(heap, (load + int(deg_local[v]), t))
    return np.concatenate([np.asarray(m, np.int64) for m in members])


def _prep_edges(cfg: Cfg, edge_index):
    """Balanced tile assignment; edges partitioned by dst owner; per dst-tile
    padded chunk schedule with table rows in group-major collective layout."""
    src = np.asarray(edge_index[0], np.int64)
    dst = np.asarray(edge_index[1], np.int64)
    deg = np.bincount(dst, minlength=cfg.N).astype(np.float64)
    rdeg_full = (1.0 / np.maximum(deg, 1.0)).astype(np.float32) / F8SCALE

    NLOC, NT = cfg.NLOC, cfg.NT
    # per-core balanced permutation: perm[c][p] = local node at tile-major pos p
    perms, poss = [], []
    for c in range(cfg.NC):
        dl = deg[c * NLOC : (c + 1) * NLOC]
        perm = _balance_tiles(dl, NT)
        pos = np.empty(NLOC, np.int64)
        pos[perm] = np.arange(NLOC)
        perms.append(perm)
        poss.append(pos)

    # table row of global node v: half-split layout. srcs with pos < HLF live
    # in table A at row c*HLF + pos; the rest in table B at c*HLF + (pos-HLF).
    HLF = cfg.HLF
    src_c = src // NLOC
    src_p = np.concatenate([poss[c][None, :] for c in range(cfg.NC)], axis=0)[
        src_c, src % NLOC
    ]
    in_a = src_p < HLF
    src_row = np.where(in_a, src_c * HLF + src_p, src_c * (NLOC - HLF) + src_p - HLF)
    assert src_row.max() < 32768

    per_core = []
    cka_max = ckb_max = 1
    for c in range(cfg.NC):
        lo, hi = c * NLOC, (c + 1) * NLOC
        m = (dst >= lo) & (dst < hi)
        s_row = src_row[m]
        s_a = in_a[m]
        d_pos = poss[c][dst[m] - lo]
        # order by (dst tile, B-half flag) so each tile lists A edges then B
        order = np.lexsort((~s_a, d_pos // 128))
        s_row, d_pos, s_a = s_row[order], d_pos[order], s_a[order]
        tb = np.searchsorted(d_pos // 128, np.arange(NT + 1))
        cnta = np.zeros(NT, np.int64)
        cntb = np.zeros(NT, np.int64)
        for t in range(NT):
            seg = s_a[tb[t] : tb[t + 1]]
            cnta[t] = int(seg.sum())
            cntb[t] = len(seg) - cnta[t]
        cka_max = max(cka_max, int(math.ceil(cnta.max() / 128)) or 1)
        ckb_max = max(ckb_max, int(math.ceil(cntb.max() / 128)) or 1)
        per_core.append((s_row, d_pos, s_a, tb, cnta, cntb))

    CKA, CKB = cka_max, ckb_max
    CK = CKA + CKB
    outs = []
    for c in range(cfg.NC):
        s_row, d_pos, s_a, tb, cnta, cntb = per_core[c]
        idx16 = np.zeros((128, NT * CK * 8), np.int16)
        ids = np.full((128, NT * CK), 255.0, BF16)
        for t in range(NT):
            na, nb = int(cnta[t]), int(cntb[t])
            for (n, roff, coff) in ((na, tb[t], 0), (nb, tb[t] + na, CKA)):
                if n == 0:
                    continue
                i = np.arange(n)
                # gather order: unwrapped[i] = idx16[i%16, i//16] (replicated x8)
                col = (t * CK + coff) * 8 + i // 16
                row = i % 16
                sv = s_row[roff : roff + n]
                for g in range(8):
                    idx16[row + 16 * g, col] = sv
                ids[i % 128, t * CK + coff + i // 128] = (
                    d_pos[roff : roff + n] - t * 128
                ).astype(BF16)
        rdeg = np.ones((128, NT), np.float32) / F8SCALE
        rfull = rdeg_full[c * NLOC : (c + 1) * NLOC][perms[c]]
        for t in range(NT):
            r0 = t * 128
            nr = min(128, NLOC - r0)
            rdeg[:nr, t] = rfull[r0 : r0 + nr]
        outs.append(dict(idx16=idx16, ids=ids, rdeg=rdeg))
    return (CKA, CKB), outs, perms


def _prep_inputs(cfg: Cfg, inputs):
    """Build the per-core in_maps. Returns ((CKA, CKB), in_maps, perms)."""
    f32 = np.float32
    x = np.asarray(inputs["x"], f32)
    CK, edge_outs, perms = _prep_edges(cfg, np.asarray(inputs["edge_index"]))

    g = lambda k: np.asarray(inputs[k], f32)

    shared = {
        "w_in_r": _prep_rhs(g("w_in")),
        "b_in_bc": _prep_bias(g("b_in")),
        "w_si_r": _prep_rhs(g("se_in_w")),
        "b_si_bc": _prep_bias(g("se_in_b")),
        "w_s1_r": np.concatenate([_prep_rhs(g("sage_w1")[i]) for i in range(cfg.NSAGE)], axis=1),
        "b_s1_bc": np.concatenate([_prep_bias(g("sage_b1")[i]) for i in range(cfg.NSAGE)], axis=1),
        "w_s2_r": np.concatenate([_prep_rhs(g("sage_w2")[i]) for i in range(cfg.NSAGE)], axis=1),
        "b_s2_bc": np.concatenate([_prep_bias(g("sage_b2")[i]) for i in range(cfg.NSAGE)], axis=1),
        "w_so_r": _prep_rhs(g("se_out_w")),
        "b_so_bc": _prep_bias(g("se_out_b")),
        "w_e1_r": np.concatenate([_prep_rhs(g("enc_w1")[k]) for k in range(cfg.NL)], axis=1),
        "b_e1_bc": np.concatenate([_prep_bias(g("enc_b1")[k]) for k in range(cfg.NL)], axis=1),
        "w_e2_r": np.concatenate(
            [_prep_rhs(np.ascontiguousarray(g("enc_w2")[k][:, 2::4])) for k in range(cfg.NL)], axis=1
        ),
        "b_e2_bc": np.concatenate([_prep_bias(g("enc_b2")[k][2::4]) for k in range(cfg.NL)], axis=1),
        "ln_g_bc": np.concatenate([_prep_bias(g("ln_g")[k]) for k in range(cfg.NL)], axis=1),
        "ln_b_bc": np.concatenate([_prep_bias(g("ln_b")[k]) for k in range(cfg.NL)], axis=1),
        "w_b1_r": np.concatenate([_prep_rhs(g("bdl_w1")[k]) for k in range(cfg.NL)], axis=1),
        "b_b1_bc": np.concatenate([_prep_bias(g("bdl_b1")[k]) for k in range(cfg.NL)], axis=1),
        "w_b2_r": np.concatenate([_prep_rhs(g("bdl_w2")[k]) for k in range(cfg.NL)], axis=1),
        "b_b2_bc": np.concatenate([_prep_bias(g("bdl_b2")[k]) for k in range(cfg.NL)], axis=1),
        "oln_g_bc": _prep_bias(g("out_ln_g")),
        "oln_b_bc": _prep_bias(g("out_ln_b")),
        "w_o_r": _prep_rhs(g("w_out")),
        "b_o_bc": _prep_bias(g("b_out")),
        "ident_b": np.eye(128, dtype=BF16),
        "iota_f": np.tile(np.arange(128), (128, 1)).astype(BF16),
    }

    in_maps = []
    for c in range(cfg.NC):
        m = dict(shared)
        m["x_c"] = np.ascontiguousarray(x[c * cfg.NLOC : (c + 1) * cfg.NLOC][perms[c]])
        m["idx16"] = edge_outs[c]["idx16"]
        m["ids_f"] = edge_outs[c]["ids"]
        m["rdeg"] = edge_outs[c]["rdeg"]
        in_maps.append(m)
    return CK, in_maps, perms


# ---------------------------------------------------------------- builder


def build_program(cfg: Cfg, CKA: int, CKB: int):
    from concourse import bacc, mybir
    import concourse.tile as tile

    f32 = mybir.dt.float32
    bf16 = mybir.dt.bfloat16
    fp8 = mybir.dt.float8e4
    i16 = mybir.dt.int16
    ALU = mybir.AluOpType
    AX = mybir.AxisListType
    ACT = mybir.ActivationFunctionType

    NT, NLOC, HID, SW = cfg.NT, cfg.NLOC, cfg.HID, cfg.SW
    CK = CKA + CKB
    HLF, HT = cfg.HLF, cfg.HT

    nc = bacc.Bacc(
        "TRN2",
        target_bir_lowering=False,
        debug=False,
        enable_asserts=False,
        num_devices=cfg.NC,
        num_swdge_queues=4,
    )
    rg = [list(range(cfg.NC))]

    # ---- external I/O
    d_x = nc.dram_tensor("x_c", [NLOC, HID], f32, kind="ExternalInput").ap()
    d_idx = nc.dram_tensor("idx16", [128, NT * CK * 8], i16, kind="ExternalInput").ap()
    d_ids = nc.dram_tensor("ids_f", [128, NT * CK], bf16, kind="ExternalInput").ap()
    d_rdeg = nc.dram_tensor("rdeg", [128, NT], f32, kind="ExternalInput").ap()

    def din(name, shape, dt):
        return nc.dram_tensor(name, shape, dt, kind="ExternalInput").ap()

    NS, NL = cfg.NSAGE, cfg.NL
    d_w_in = din("w_in_r", [128, 2 * HID], bf16)
    d_b_in = din("b_in_bc", [128, HID], f32)
    d_w_si = din("w_si_r", [128, 2 * SW], bf16)
    d_b_si = din("b_si_bc", [128, SW], f32)
    d_w_s1 = din("w_s1_r", [128, NS * 8 * SW], bf16)
    d_b_s1 = din("b_s1_bc", [128, NS * SW], f32)
    d_w_s2 = din("w_s2_r", [128, NS * 4 * SW], bf16)
    d_b_s2 = din("b_s2_bc", [128, NS * SW], f32)
    d_w_so = din("w_so_r", [128, 4 * SW], bf16)
    d_b_so = din("b_so_bc", [128, SW], f32)
    d_w_e1 = din("w_e1_r", [128, NL * 4 * SW], bf16)
    d_b_e1 = din("b_e1_bc", [128, NL * SW], f32)
    d_w_e2 = din("w_e2_r", [128, NL * 4 * 128], bf16)
    d_b_e2 = din("b_e2_bc", [128, NL * 128], f32)
    d_ln_g = din("ln_g_bc", [128, NL * HID], f32)
    d_ln_b = din("ln_b_bc", [128, NL * HID], f32)
    d_w_b1 = din("w_b1_r", [128, NL * 4 * HID], bf16)
    d_b_b1 = din("b_b1_bc", [128, NL * HID], f32)
    d_w_b2 = din("w_b2_r", [128, NL * 2 * HID], bf16)
    d_b_b2 = din("b_b2_bc", [128, NL * HID], f32)
    d_oln_g = din("oln_g_bc", [128, HID], f32)
    d_oln_b = din("oln_b_bc", [128, HID], f32)
    d_w_o = din("w_o_r", [128, 2 * cfg.OUT], bf16)
    d_b_o = din("b_o_bc", [128, cfg.OUT], f32)
    d_identb = din("ident_b", [128, 128], bf16)
    d_iota = din("iota_f", [128, 128], bf16)

    d_out = nc.dram_tensor("out", [NLOC, cfg.OUT], f32, kind="ExternalOutput").ap()

    with tile.TileContext(nc) as tc:
        from contextlib import ExitStack

        ctx = ExitStack()
        pers = ctx.enter_context(tc.tile_pool(name="pers", bufs=1))
        wout = ctx.enter_context(tc.tile_pool(name="wout", bufs=1))
        wsage = ctx.enter_context(tc.tile_pool(name="wsage", bufs=1))
        work = ctx.enter_context(tc.tile_pool(name="work", bufs=2))
        small = ctx.enter_context(tc.tile_pool(name="small", bufs=2))
        spool = ctx.enter_context(tc.tile_pool(name="spool", bufs=2))
        zgp = ctx.enter_context(tc.tile_pool(name="zgp", bufs=2))
        dram = ctx.enter_context(tc.tile_pool(name="dram", bufs=2, space="DRAM"))
        psum = ctx.enter_context(tc.tile_pool(name="psum", bufs=2, space="PSUM"))

        # ---- persistent SBUF residents
        identb = pers.tile([128, 128], bf16, name="identb")
        iota = pers.tile([128, 128], bf16, name="iota")
        rdeg = pers.tile([128, NT], f32, name="rdegs")
        ids = pers.tile([128, NT * CK], bf16, name="idss")
        idx = pers.tile([128, NT * CK * 8], i16, name="idxs")
        nc.sync.dma_start(out=identb[:], in_=d_identb[:])
        nc.sync.dma_start(out=iota[:], in_=d_iota[:])
        nc.sync.dma_start(out=rdeg[:], in_=d_rdeg[:])
        nc.sync.dma_start(out=ids[:], in_=d_ids[:])
        nc.sync.dma_start(out=idx[:], in_=d_idx[:])

        h_t = [pers.tile([128, HID], f32, name=f"h{t}") for t in range(NT)]
        z_t = [pers.tile([128, SW], bf16, name=f"z{t}") for t in range(NT)]
        c_t = [pers.tile([128, 128], bf16, name=f"rc{t}") for t in range(NT)]
        s_t = [pers.tile([128, 128], bf16, name=f"rs{t}") for t in range(NT)]

        def load_w(pool, name, src, cols, dt):
            t = pool.tile([128, cols], dt, tag=name, name=name)
            nc.sync.dma_start(out=t[:], in_=src)
            return t

        def dram_tile(name, shape, dt, shared=False):
            return dram.tile(
                shape, dt, tag=name, name=name, addr_space="Shared" if shared else "Local"
            )

        def rows_of(t):
            return min(128, NLOC - t * 128)

        # ---------- helpers ----------
        def transpose_into(dst, src_ap, nchunks):
            """dst[:, kc*128:(kc+1)*128] = src[:, kc*128:(kc+1)*128]^T (bf16).
            All chunks land in one PSUM bank (start zero-fills the region),
            evacuated with a single vector copy."""
            tp = psum.tile([128, 4 * 128], bf16, tag="tr", name="trb")
            for kc in range(nchunks):
                nc.tensor.matmul(
                    tp[:, kc * 128 : (kc + 1) * 128],
                    lhsT=src_ap[:, kc * 128 : (kc + 1) * 128],
                    rhs=identb[:],
                    is_transpose=True,
                    start=(kc == 0),
                    stop=(kc == nchunks - 1),
                )
            nc.vector.tensor_copy(out=dst[:, : nchunks * 128], in_=tp[:, : nchunks * 128])

        def mm_acc(ps_ap, lhsT_tile, rhs_tile, kcs, m, rhs_block, preloaded=False):
            """ps (+)= sum_kc lhsT[:, kc]^T @ rhs[:, kc-block] (node-major out)."""
            for kc in range(kcs):
                nc.tensor.matmul(
                    ps_ap,
                    lhsT=lhsT_tile[:, kc * 128 : (kc + 1) * 128],
                    rhs=rhs_tile[:, kc * rhs_block + m[0] : kc * rhs_block + m[1]],
                    start=(kc == 0 and not preloaded),
                    stop=(kc == kcs - 1),
                    skip_group_check=preloaded,
                )

        def emit_ln(h_ap, g_bc_ap, b_bc_ap, out_ap, w):
            """LayerNorm with row stats on the Scalar engine."""
            junk = work.tile([128, w], f32, tag="lnjunk", name="lnjunk")
            s1 = small.tile([128, 1], f32, tag="ln1", name="ln1")
            nc.scalar.activation(out=junk[:], in_=h_ap, func=ACT.Identity, accum_out=s1[:])
            nm = small.tile([128, 1], f32, tag="ln2", name="ln2")
            nc.vector.tensor_scalar(nm[:], s1[:], -1.0 / w, None, ALU.mult)
            junk2 = work.tile([128, w], f32, tag="lnjunk2", name="lnjunk2")
            v = small.tile([128, 1], f32, tag="ln3", name="ln3")
            nc.scalar.activation(
                out=junk2[:], in_=h_ap, func=ACT.Square, bias=nm[:], accum_out=v[:]
            )
            vm = small.tile([128, 1], f32, tag="ln4", name="ln4")
            nc.vector.tensor_scalar(vm[:], v[:], 1.0 / w, cfg.EPS, ALU.mult, ALU.add)
            r = small.tile([128, 1], f32, tag="ln5", name="ln5")
            nc.vector.reciprocal(out=r[:], in_=vm[:])
            rs = small.tile([128, 1], f32, tag="ln6", name="ln6")
            nc.scalar.sqrt(out=rs[:], in_=r[:])
            nmrs = small.tile([128, 1], f32, tag="ln7", name="ln7")
            nc.vector.tensor_tensor(out=nmrs[:], in0=nm[:], in1=rs[:], op=ALU.mult)
            hn0 = work.tile([128, w], bf16, tag="lnhn0", name="lnhn0")
            nc.scalar.activation(
                out=hn0[:], in_=h_ap, func=ACT.Identity, bias=nmrs[:], scale=rs[:]
            )
            hn1 = work.tile([128, w], bf16, tag="lnhn1", name="lnhn1")
            nc.vector.tensor_tensor(out=hn1[:], in0=hn0[:], in1=g_bc_ap, op=ALU.mult)
            nc.vector.tensor_tensor(out=out_ap, in0=hn1[:], in1=b_bc_ap, op=ALU.add)

        def emit_agg(tabA_ap, tabB_ap, t, width, ps_ap):
            """Gather + one-hot matmul segment sum for dst tile t into psum.
            A-half chunks (queues 0/1) read tabA; B-half (queues 2/3) tabB."""
            zg = zgp.tile([128, CK, width], fp8, tag="zg", name="zg")
            ca2, cb2 = (CKA + 1) // 2, (CKB + 1) // 2
            calls = (
                (0, 0, ca2, tabA_ap),
                (1, ca2, CKA, tabA_ap),
                (2, CKA, CKA + cb2, tabB_ap),
                (3, CKA + cb2, CK, tabB_ap),
            )
            for qi, c0, c1, tab in calls:
                if c1 > c0:
                    nc.gpsimd.dma_gather(
                        out_ap=zg[:, c0:c1, :],
                        in_ap=tab,
                        idxs_ap=idx[:, t * CK * 8 + c0 * 8 : t * CK * 8 + c1 * 8],
                        num_idxs=(c1 - c0) * 128,
                        num_idxs_reg=(c1 - c0) * 128,
                        elem_size=width,
                        single_packet=False,
                        queue_num=qi,
                    )
            Sall = spool.tile([128, CK * 128], bf16, tag="S", name="S")
            iota_bc = iota[:].rearrange("p (o f) -> p o f", o=1).to_broadcast([128, CK, 128])
            ids_bc = (
                ids[:, t * CK : (t + 1) * CK]
                .rearrange("p (c o) -> p c o", o=1)
                .to_broadcast([128, CK, 128])
            )
            nc.vector.tensor_tensor(out=Sall[:], in0=iota_bc, in1=ids_bc, op=ALU.is_equal)
            for c in range(CK):
                nc.tensor.matmul(
                    ps_ap,
                    lhsT=Sall[:, c * 128 : (c + 1) * 128],
                    rhs=zg[:, c, :],
                    start=(c == 0),
                    stop=(c == CK - 1),
                )

        def fire_halves(t, loc_ap, tabA_ap, tabB_ap):
            """Issue the half-table AllGathers as their rows complete."""
            if t == HT - 1:
                nc.gpsimd.collective_compute(
                    "AllGather", ALU.bypass, replica_groups=rg,
                    ins=[loc_ap[0:HLF, :]], outs=[tabA_ap],
                )
            elif t == NT - 1:
                nc.gpsimd.collective_compute(
                    "AllGather", ALU.bypass, replica_groups=rg,
                    ins=[loc_ap[HLF:NLOC, :]], outs=[tabB_ap],
                )

        def emit_z0(t, w_si_sb, b_si_sb, loc8_ap, tabA_ap, tabB_ap, hb16):
            """z0 = gelu(h@w_si+b_si) -> z_t[t] (bf16) + fp8 table row write."""
            nr = rows_of(t)
            hT = work.tile([128, 2 * 128], bf16, tag="hT", name="hT")
            transpose_into(hT, hb16[:], 2)
            zp = psum.tile([128, SW], f32, tag="mlp", name="zp")
            nc.scalar.copy(out=zp[:], in_=b_si_sb[:])
            mm_acc(zp[:], hT, w_si_sb, 2, (0, SW), SW, preloaded=True)
            nc.scalar.activation(out=z_t[t][:], in_=zp[:], func=ACT.Gelu)
            z8 = work.tile([128, SW], fp8, tag="z8", name="z8")
            nc.vector.tensor_scalar(z8[:], z_t[t][:], F8SCALE, None, ALU.mult)
            nc.sync.dma_start(out=loc8_ap[t * 128 : t * 128 + nr, :], in_=z8[:nr, :])
            fire_halves(t, loc8_ap, tabA_ap, tabB_ap)

        # ================= phase 0: h0 = gelu(x @ w_in + b_in), z0 =================
        w_in_sb = load_w(wout, "w_in", d_w_in[:], 2 * HID, bf16)
        b_in_sb = load_w(wout, "b_in", d_b_in[:], HID, f32)
        w_si_sb = load_w(wout, "w_si", d_w_si[:], 2 * SW, bf16)
        b_si_sb = load_w(wout, "b_si", d_b_si[:], SW, f32)

        loc8 = dram_tile("loc8", [NLOC, SW], fp8)
        tabA = dram_tile("tabA", [cfg.NC * HLF, SW], fp8, shared=True)
        tabB = dram_tile("tabB", [cfg.NC * (NLOC - HLF), SW], fp8, shared=True)
        for t in range(NT):
            nr = rows_of(t)
            xt = work.tile([128, HID], f32, tag="xt", name="xt")
            if nr < 128:
                nc.gpsimd.memset(xt[:], 0.0)
            nc.sync.dma_start(out=xt[:nr, :], in_=d_x[t * 128 : t * 128 + nr, :])
            xb = work.tile([128, HID], bf16, tag="hb16", name="xb")
            nc.vector.tensor_copy(out=xb[:], in_=xt[:])
            xT = work.tile([128, 2 * 128], bf16, tag="xT", name="xT")
            transpose_into(xT, xb[:], 2)
            hp = psum.tile([128, HID], f32, tag="mlp", name="hp")
            nc.scalar.copy(out=hp[:], in_=b_in_sb[:])
            mm_acc(hp[:], xT, w_in_sb, 2, (0, HID), HID, preloaded=True)
            nc.scalar.activation(out=h_t[t][:], in_=hp[:], func=ACT.Gelu)
            hb16 = work.tile([128, HID], bf16, tag="hb16", name="hb16")
            nc.vector.tensor_copy(out=hb16[:], in_=h_t[t][:])
            emit_z0(t, w_si_sb, b_si_sb, loc8, tabA, tabB, hb16)

        # ================= outer layers =================
        for k in range(NL):
            # ---- per-outer weights
            w_so_sb = load_w(wout, "w_so", d_w_so[:], 4 * SW, bf16)
            b_so_sb = load_w(wout, "b_so", d_b_so[:], SW, f32)
            w_e1_sb = load_w(wout, "w_e1", d_w_e1[:, k * 4 * SW : (k + 1) * 4 * SW], 4 * SW, bf16)
            b_e1_sb = load_w(wout, "b_e1", d_b_e1[:, k * SW : (k + 1) * SW], SW, f32)
            w_e2_sb = load_w(wout, "w_e2", d_w_e2[:, k * 4 * 128 : (k + 1) * 4 * 128], 4 * 128, bf16)
            b_e2_sb = load_w(wout, "b_e2", d_b_e2[:, k * 128 : (k + 1) * 128], 128, f32)
            ln_g_sb = load_w(wout, "ln_g", d_ln_g[:, k * HID : (k + 1) * HID], HID, f32)
            ln_b_sb = load_w(wout, "ln_b", d_ln_b[:, k * HID : (k + 1) * HID], HID, f32)
            w_b1_sb = load_w(wout, "w_b1", d_w_b1[:, k * 4 * HID : (k + 1) * 4 * HID], 4 * HID, bf16)
            b_b1_sb = load_w(wout, "b_b1", d_b_b1[:, k * HID : (k + 1) * HID], HID, f32)
            w_b2_sb = load_w(wout, "w_b2", d_w_b2[:, k * 2 * HID : (k + 1) * 2 * HID], 2 * HID, bf16)
            b_b2_sb = load_w(wout, "b_b2", d_b_b2[:, k * HID : (k + 1) * HID], HID, f32)

            tabA_prev, tabB_prev = tabA, tabB
            locy8 = dram_tile("locy8", [NLOC, HID], fp8)
            ytabA = dram_tile("ytabA", [cfg.NC * HLF, HID], fp8, shared=True)
            ytabB = dram_tile("ytabB", [cfg.NC * (NLOC - HLF), HID], fp8, shared=True)

            # ---- SAGE layers
            for i in range(cfg.NSAGE):
                w1_sb = load_w(wsage, "w1", d_w_s1[:, i * 8 * SW : (i + 1) * 8 * SW], 8 * SW, bf16)
                b1_sb = load_w(wsage, "b1", d_b_s1[:, i * SW : (i + 1) * SW], SW, f32)
                w2_sb = load_w(wsage, "w2", d_w_s2[:, i * 4 * SW : (i + 1) * 4 * SW], 4 * SW, bf16)
                b2_sb = load_w(wsage, "b2", d_b_s2[:, i * SW : (i + 1) * SW], SW, f32)
                last = i == cfg.NSAGE - 1
                if not last:
                    loc8_cur = dram_tile("loc8", [NLOC, SW], fp8)
                    tabA_cur = dram_tile("tabA", [cfg.NC * HLF, SW], fp8, shared=True)
                    tabB_cur = dram_tile("tabB", [cfg.NC * (NLOC - HLF), SW], fp8, shared=True)

                for t in range(NT):
                    nr = rows_of(t)
                    # aggregation from previous table
                    aps = psum.tile([128, SW], f32, tag="agg", name="aps")
                    emit_agg(tabA_prev[:], tabB_prev[:], t, SW, aps[:])
                    m_sb = work.tile([128, SW], bf16, tag="msb", name="msb")
                    nc.scalar.mul(out=m_sb[:], in_=aps[:], mul=rdeg[:, t : t + 1])
                    # zc^T = [z | m]^T
                    zcT = work.tile([128, 8 * 128], bf16, tag="zcT", name="zcT")
                    transpose_into(zcT[:, : 4 * 128], z_t[t][:], 4)
                    transpose_into(zcT[:, 4 * 128 : 8 * 128], m_sb[:], 4)
                    # MLP1
                    p1p = psum.tile([128, SW], f32, tag="mlp", name="p1p")
                    nc.scalar.copy(out=p1p[:], in_=b1_sb[:])
                    mm_acc(p1p[:], zcT, w1_sb, 8, (0, SW), SW, preloaded=True)
                    p1 = work.tile([128, SW], bf16, tag="p1", name="p1")
                    nc.scalar.activation(out=p1[:], in_=p1p[:], func=ACT.Gelu)
                    p1T = work.tile([128, 4 * 128], bf16, tag="p1T", name="p1T")
                    transpose_into(p1T, p1[:], 4)
                    # MLP2 + residual (z + b2 preloaded into psum)
                    p2p = psum.tile([128, SW], f32, tag="mlp", name="p2p")
                    nc.vector.tensor_tensor(out=p2p[:], in0=z_t[t][:], in1=b2_sb[:], op=ALU.add)
                    mm_acc(p2p[:], p1T, w2_sb, 4, (0, SW), SW, preloaded=True)
                    nc.scalar.copy(out=z_t[t][:], in_=p2p[:])
                    if not last:
                        z8 = work.tile([128, SW], fp8, tag="z8", name="z8")
                        nc.vector.tensor_scalar(z8[:], p2p[:], F8SCALE, None, ALU.mult)
                        nc.sync.dma_start(
                            out=loc8_cur[t * 128 : t * 128 + nr, :], in_=z8[:nr, :]
                        )
                        fire_halves(t, loc8_cur, tabA_cur, tabB_cur)
                        continue

                    # ---- fused: enc path -> rotation coefs; LN(h) -> y -> locy8
                    z5T = work.tile([128, 4 * 128], bf16, tag="z5T", name="z5T")
                    transpose_into(z5T, z_t[t][:], 4)
                    ep = psum.tile([128, SW], f32, tag="mlp", name="ep")
                    nc.scalar.copy(out=ep[:], in_=b_so_sb[:])
                    mm_acc(ep[:], z5T, w_so_sb, 4, (0, SW), SW, preloaded=True)
                    enc = work.tile([128, SW], bf16, tag="enc", name="enc")
                    nc.scalar.copy(out=enc[:], in_=ep[:])
                    encT = work.tile([128, 4 * 128], bf16, tag="encT", name="encT")
                    transpose_into(encT, enc[:], 4)
                    gp = psum.tile([128, SW], f32, tag="mlp", name="gp")
                    nc.scalar.copy(out=gp[:], in_=b_e1_sb[:])
                    mm_acc(gp[:], encT, w_e1_sb, 4, (0, SW), SW, preloaded=True)
                    gact = work.tile([128, SW], bf16, tag="gact", name="gact")
                    nc.scalar.activation(out=gact[:], in_=gp[:], func=ACT.Gelu)
                    gT = work.tile([128, 4 * 128], bf16, tag="gT", name="gT")
                    transpose_into(gT, gact[:], 4)
                    ap_ = psum.tile([128, 128], f32, tag="agg", name="ap_")
                    nc.scalar.copy(out=ap_[:], in_=b_e2_sb[:])
                    mm_acc(ap_[:], gT, w_e2_sb, 4, (0, 128), 128, preloaded=True)
                    a_sb = work.tile([128, 128], f32, tag="a0", name="a_sb")
                    nc.scalar.copy(out=a_sb[:], in_=ap_[:])
                    a2 = work.tile([128, 128], f32, tag="a1", name="a2")
                    nc.vector.tensor_tensor(out=a2[:], in0=a_sb[:], in1=a_sb[:], op=ALU.mult)
                    rinv = work.tile([128, 128], f32, tag="a2t", name="rinv")
                    nc.vector.tensor_scalar(rinv[:], a2[:], 1.0, None, ALU.add)
                    nc.vector.reciprocal(out=rinv[:], in_=rinv[:])
                    nc.vector.tensor_scalar(a2[:], a2[:], -1.0, None, ALU.add)
                    nc.vector.tensor_tensor(out=c_t[t][:], in0=a2[:], in1=rinv[:], op=ALU.mult)
                    nc.vector.tensor_scalar(a_sb[:], a_sb[:], 2.0, None, ALU.mult)
                    nc.vector.tensor_tensor(out=s_t[t][:], in0=a_sb[:], in1=rinv[:], op=ALU.mult)

                    # LN(h) -> hn; y = rot(hn); y8 = y*16 fp8
                    hn = work.tile([128, HID], bf16, tag="hn", name="hn")
                    emit_ln(h_t[t][:], ln_g_sb[:], ln_b_sb[:], hn[:], HID)
                    hn_ev = hn[:, 0:HID:2]
                    hn_od = hn[:, 1:HID:2]
                    y = work.tile([128, HID], bf16, tag="y", name="y")
                    t0 = work.tile([128, 128], bf16, tag="r0", name="t0")
                    t1 = work.tile([128, 128], bf16, tag="r1", name="t1")
                    nc.vector.tensor_tensor(out=t0[:], in0=c_t[t][:], in1=hn_ev, op=ALU.mult)
                    nc.vector.tensor_tensor(out=t1[:], in0=s_t[t][:], in1=hn_od, op=ALU.mult)
                    nc.vector.tensor_tensor(out=y[:, 0:HID:2], in0=t0[:], in1=t1[:], op=ALU.add)
                    nc.vector.tensor_tensor(out=t0[:], in0=c_t[t][:], in1=hn_od, op=ALU.mult)
                    nc.vector.tensor_tensor(out=t1[:], in0=s_t[t][:], in1=hn_ev, op=ALU.mult)
                    nc.vector.tensor_tensor(
                        out=y[:, 1:HID:2], in0=t0[:], in1=t1[:], op=ALU.subtract
                    )
                    y8 = work.tile([128, HID], fp8, tag="y8", name="y8")
                    nc.vector.tensor_scalar(y8[:], y[:], F8SCALE, None, ALU.mult)
                    nc.sync.dma_start(
                        out=locy8[t * 128 : t * 128 + nr, :], in_=y8[:nr, :]
                    )
                    fire_halves(t, locy8, ytabA, ytabB)

                if not last:
                    tabA_prev, tabB_prev = tabA_cur, tabB_cur
                    loc8 = loc8_cur

            # ---- BDL message + MLP, h update (+ fused z0 of next layer / output)
            if k + 1 < NL:
                loc8_nxt = dram_tile("loc8", [NLOC, SW], fp8)
                tabA_nxt = dram_tile("tabA", [cfg.NC * HLF, SW], fp8, shared=True)
                tabB_nxt = dram_tile("tabB", [cfg.NC * (NLOC - HLF), SW], fp8, shared=True)
            else:
                oln_g_sb = load_w(wout, "oln_g", d_oln_g[:], HID, f32)
                oln_b_sb = load_w(wout, "oln_b", d_oln_b[:], HID, f32)
                w_o_sb = load_w(wout, "w_o", d_w_o[:], 2 * cfg.OUT, bf16)
                b_o_sb = load_w(wout, "b_o", d_b_o[:], cfg.OUT, f32)

            for t in range(NT):
                nr = rows_of(t)
                yps = psum.tile([128, HID], f32, tag="agg", name="yps")
                emit_agg(ytabA[:], ytabB[:], t, HID, yps[:])
                ga = work.tile([128, HID], bf16, tag="ga", name="ga")
                nc.vector.tensor_scalar(ga[:], yps[:], rdeg[:, t : t + 1], None, ALU.mult)
                # hn and msg (rotated back aggregate)
                hn = work.tile([128, HID], bf16, tag="hn", name="hnb")
                emit_ln(h_t[t][:], ln_g_sb[:], ln_b_sb[:], hn[:], HID)
                g_ev = ga[:, 0:HID:2]
                g_od = ga[:, 1:HID:2]
                msg = work.tile([128, HID], bf16, tag="msg", name="msg")
                t0 = work.tile([128, 128], bf16, tag="r0", name="t0b")
                t1 = work.tile([128, 128], bf16, tag="r1", name="t1b")
                nc.vector.tensor_tensor(out=t0[:], in0=c_t[t][:], in1=g_ev, op=ALU.mult)
                nc.vector.tensor_tensor(out=t1[:], in0=s_t[t][:], in1=g_od, op=ALU.mult)
                nc.vector.tensor_tensor(
                    out=msg[:, 0:HID:2], in0=t0[:], in1=t1[:], op=ALU.subtract
                )
                nc.vector.tensor_tensor(out=t0[:], in0=s_t[t][:], in1=g_ev, op=ALU.mult)
                nc.vector.tensor_tensor(out=t1[:], in0=c_t[t][:], in1=g_od, op=ALU.mult)
                nc.vector.tensor_tensor(
                    out=msg[:, 1:HID:2], in0=t0[:], in1=t1[:], op=ALU.add
                )
                hcT = work.tile([128, 4 * 128], bf16, tag="hcT", name="hcT")
                transpose_into(hcT[:, : 2 * 128], hn[:], 2)
                transpose_into(hcT[:, 2 * 128 : 4 * 128], msg[:], 2)
                bp = psum.tile([128, HID], f32, tag="mlp", name="bp")
                nc.scalar.copy(out=bp[:], in_=b_b1_sb[:])
                mm_acc(bp[:], hcT, w_b1_sb, 4, (0, HID), HID, preloaded=True)
                tb = work.tile([128, HID], bf16, tag="tb", name="tb")
                nc.scalar.activation(out=tb[:], in_=bp[:], func=ACT.Gelu)
                tbT = work.tile([128, 2 * 128], bf16, tag="tbT", name="tbT")
                transpose_into(tbT, tb[:], 2)
                b2p = psum.tile([128, HID], f32, tag="mlp", name="b2p")
                nc.vector.tensor_tensor(out=b2p[:], in0=h_t[t][:], in1=b_b2_sb[:], op=ALU.add)
                mm_acc(b2p[:], tbT, w_b2_sb, 2, (0, HID), HID, preloaded=True)
                nc.vector.tensor_copy(out=h_t[t][:], in_=b2p[:])

                if k + 1 < NL:
                    hb16 = work.tile([128, HID], bf16, tag="hb16", name="hb16b")
                    nc.vector.tensor_copy(out=hb16[:], in_=h_t[t][:])
                    emit_z0(t, w_si_sb, b_si_sb, loc8_nxt, tabA_nxt, tabB_nxt, hb16)
                else:
                    hnf = work.tile([128, HID], bf16, tag="hn", name="hnf")
                    emit_ln(h_t[t][:], oln_g_sb[:], oln_b_sb[:], hnf[:], HID)
                    hnfT = work.tile([128, 2 * 128], bf16, tag="tbT", name="hnfT")
                    transpose_into(hnfT, hnf[:], 2)
                    op_ = psum.tile([128, cfg.OUT], f32, tag="mlp", name="op_")
                    nc.scalar.copy(out=op_[:], in_=b_o_sb[:])
                    mm_acc(op_[:], hnfT, w_o_sb, 2, (0, cfg.OUT), cfg.OUT, preloaded=True)
                    ot = work.tile([128, cfg.OUT], f32, tag="ot", name="ot")
                    nc.scalar.copy(out=ot[:], in_=op_[:])
                    nc.sync.dma_start(out=d_out[t * 128 : t * 128 + nr, :], in_=ot[:nr, :])

            if k + 1 < NL:
                tabA, tabB = tabA_nxt, tabB_nxt
                loc8 = loc8_nxt

        ctx.close()

    nc.compile()
    return nc


# ---------------------------------------------------------------- runner

_CACHE = {}


def _get_program(cfg: Cfg, CKA: int, CKB: int):
    key = (cfg, CKA, CKB)
    if key not in _CACHE:
        _CACHE[key] = build_program(cfg, CKA, CKB)
    return _CACHE[key]


def run(inputs, cfg: Cfg = CFG, trace: bool = False):
    from concourse import bass_utils

    (CKA, CKB), in_maps, perms = _prep_inputs(cfg, inputs)
    nc = _get_program(cfg, CKA, CKB)
    res = bass_utils.run_bass_kernel_spmd(
        nc, in_maps, core_ids=list(range(cfg.NC)), trace=trace
    )
    out = np.empty((cfg.N, cfg.OUT), np.float32)
    for c in range(cfg.NC):
        out[c * cfg.NLOC + perms[c]] = np.asarray(res.results[c]["out"])
    return out, res


def kernel(**inputs):
    out, _ = run(inputs)
    return out
